# revision 60
# baseline (speedup 1.0000x reference)
"""Trainium2 Bass kernel for a 2-layer GNN message-passing block (SAGE-style).

Computation (see reference):
    h1 = x @ W1_root + seg_sum(x[src], dst) @ W1_nbr + b1
    a2 = seg_sum(h1[src], dst) / max(deg, 1)
    h2 = h1 @ W2_root + a2 @ W2_nbr + b2
    out = relu(h1 @ lin_w[:, :D].T + h2 @ lin_w[:, D:].T + lin_b)

Sharding: nodes are dealt to (core, 128-node group) slots in descending
in-degree order (snake), which balances per-group edge counts across cores;
edges are sharded by destination so the segment reduction is device-local.

Profiling-driven design notes (the original kernel spent ~73% of its span
on SWDGE descriptor generation; the sustained dma_gather floor is ~2.5us
per 1024-descriptor instruction, DMA-round-trip bound with 4 queues):
  * Layer-1 messages x[src] are a pure gather of the *input*, so they are
    materialized host-side as a dst-sorted bf16 stream (padded to 128
    cols so LDWEIGHTS gets fast-weight-load) and read with sequential
    DMA in 4-tile slabs — no SWDGE descriptors for layer 1.
  * Everything on the PE runs in bf16 (f32 PSUM accumulation); gather
    tables are bf16 so layer-2 SWDGE gathers move 256B/edge.
  * Layer-2 keeps the SWDGE dma_gather path (h1 is device-computed): 4
    queues round-robin, edges host-sorted by destination, segment sum via
    one-hot matmuls into one PSUM tile per node group (dma_scatter_add
    loses colliding row updates on HW, so no scatter is used). 1/deg is
    a pure function of edge_index, so it is computed host-side.
  * Layer-1's one-hots are *constant*: nodes are dealt round-robin by
    in-degree rank, so rows within a group are degree-sorted and the
    host packs each dst's edges into fixed 8-slot blocks (16 dsts per
    chunk). One constant pattern tile replaces all per-tile DVE
    is_equal builds, which previously paced layer 1.
  * h1 lives in two tables (30/19 groups; the int16 gather-index limit
    caps a table at 31 groups x 8 cores). Each AllGather fires as soon
    as its table is written; the asymmetric split makes pass 0 long
    enough to hide the second AllGather. Layer 2 runs as K=2 passes;
    pass-0 partial sums park in SBUF (bf16) and are injected back into
    PSUM via an identity matmul during pass 1. Deep (24-buffer) msg
    tiles let gathers run well ahead of their consuming matmuls.
  * All index/dst-value arrays are uploaded in partition-major layouts
    and loaded with one big DMA each.

Dense math runs feature-major: weights load as stationary lhsT once and
node columns stream as rhs. The final output is produced transposed and
scattered back to original node order on the host.
"""
import sys

sys.path.insert(0, "/opt/trn_rl_repo")

import numpy as np
import ml_dtypes

import concourse.bass as bass
import concourse.mybir as mybir
from concourse import bacc, tile
from concourse.bass_utils import run_bass_kernel_spmd
from concourse.masks import make_identity

F32 = mybir.dt.float32
BF16 = mybir.dt.bfloat16
I16 = mybir.dt.int16
I32 = mybir.dt.int32
NPBF = ml_dtypes.bfloat16

DEFAULT_CFG = dict(
    N=50000,      # nodes
    D=96,         # feature dim
    CORES=8,
    T1=2048,      # edge slots per layer-1 stream tile
    T2=1024,      # edge slots per layer-2 gather tile (ucode ring: <=1024)
    SLAB=4,       # layer-1 stream tiles per DMA (bigger descriptors)
    K=2,          # h1 table splits (pipelined all-gathers / layer-2 passes)
    SPLIT0=30,    # groups in table 0 (pass-0 share of layer-2 edges)
)


def _derive(cfg):
    c = dict(cfg)
    c["NPC"] = c["N"] // c["CORES"]              # nodes per core (logical)
    c["NPCP"] = -(-c["NPC"] // 128) * 128        # padded to node groups
    c["NT"] = c["NPCP"] // 128                   # node groups per core
    c["DP"] = 128                                # padded feature dim
    c["CPT1"] = c["T1"] // 128                   # chunks per layer-1 tile
    c["CPT2"] = c["T2"] // 128                   # chunks per layer-2 tile
    K = c["K"]
    if K == 2 and c.get("SPLIT0"):
        nts = [c["SPLIT0"], c["NT"] - c["SPLIT0"]]
    else:
        base, rem = divmod(c["NT"], K)
        nts = [base + (1 if k < rem else 0) for k in range(K)]
    c["GS"] = np.concatenate([[0], np.cumsum(nts)]).tolist()  # group bounds
    c["HS"] = [n * 128 for n in nts]                          # rows per core
    for h in c["HS"]:
        assert c["CORES"] * h < 32768, "int16 gather index overflow"
    return c


def _wrap_idxs(arr, n_tiles, T):
    """int arr [n_tiles*T] -> [n_tiles, 128, T//16] int16 in the SWDGE
    wrapped layout: element (t, p, s) = arr[t*T + s*16 + p%16]."""
    w = arr.reshape(n_tiles, T // 16, 16).transpose(0, 2, 1)  # [nt, 16, S]
    return np.ascontiguousarray(np.tile(w, (1, 8, 1)).astype(np.int16))


def _chunk_schedule(cnt_by_core, CPT, floor1):
    """cnt_by_core [CORES, NT] -> uniform-across-cores chunk schedule."""
    sl = (-(-cnt_by_core // 128)).max(axis=0)
    if floor1:
        # every group needs >=1 chunk so its PSUM tile is always written
        sl = np.maximum(sl, 1)
    starts = np.concatenate([[0], np.cumsum(sl)])
    tot = max(1, int(sl.sum()))
    n_tiles = -(-tot // CPT)
    tile_cpt = [min(CPT, max(1, tot - t * CPT)) for t in range(n_tiles)]
    return sl, starts, n_tiles, tile_cpt


def _prep(inputs, cfg):
    """Host-side sharding. Returns (in_maps, meta, node2row) where
    node2row[n] is the node's row in the padded per-core layout."""
    N, D, CORES, K = cfg["N"], cfg["D"], cfg["CORES"], cfg["K"]
    NPCP, NT, DP = cfg["NPCP"], cfg["NT"], cfg["DP"]
    T1, CPT1, T2, CPT2 = cfg["T1"], cfg["CPT1"], cfg["T2"], cfg["CPT2"]
    GS, HS = cfg["GS"], cfg["HS"]

    x = np.asarray(inputs["x"], np.float32)
    x_bf = x.astype(NPBF)
    ei = np.asarray(inputs["edge_index"]).astype(np.int64)
    src, dst = ei[0], ei[1]

    # deal nodes round-robin by in-degree rank: rank r -> core r%CORES,
    # per-core slot r//CORES (group-major). Every core sees the same degree
    # sequence (edge balance), and rows within a group are degree-sorted so
    # fixed-block layer-1 chunks waste little padding.
    deg_in = np.bincount(dst, minlength=N)
    order_nodes = np.argsort(-deg_in, kind="stable")
    rank = np.empty(N, np.int64)
    rank[order_nodes] = np.arange(N)
    owner_of = rank % CORES
    q_ = rank // CORES
    assert q_.max() < NPCP
    node2row = owner_of * NPCP + q_  # global padded row

    owner = owner_of[dst]
    row_d = node2row[dst]

    # per-core dst-sorted edge lists
    per_core = []
    for c in range(CORES):
        sel = owner == c
        d = row_d[sel] - c * NPCP
        order = np.argsort(d, kind="stable")
        per_core.append((src[sel][order], d[order]))

    # ---- layer-1 fixed-block schedule ----
    # chunk pattern j covers dsts 16j..16j+15 of its group, 8 slots each;
    # slice j repeats ceil(max_deg/8) times (uniform across cores because
    # the degree deal aligns slice degrees). One constant one-hot per j.
    BS, DPC = 8, 16            # slots per dst block, dsts per chunk
    # max degree per (group, slice) over all cores
    deg_row = np.zeros(CORES * NPCP, np.int64)
    deg_row[node2row] = deg_in
    deg_row = deg_row.reshape(CORES, NT, 128 // DPC, DPC)  # [c, g, slice, dst]
    nrep = np.maximum(-(-deg_row.max(axis=(0, 3)) // BS), 1)  # [NT, 8 slices]
    pat1, grp_chunks = [], []   # pattern id per global chunk, chunks per group
    for g in range(NT):
        cl = []
        for j in range(128 // DPC):
            cl += [j] * int(nrep[g, j])
        grp_chunks.append(len(cl))
        pat1 += cl
    sl1 = np.array(grp_chunks, np.int64)
    st1 = np.concatenate([[0], np.cumsum(sl1)])
    tot1 = int(sl1.sum())
    nt1 = -(-tot1 // CPT1)
    tcpt1 = [min(CPT1, max(1, tot1 - t * CPT1)) for t in range(nt1)]

    # ---- layer-2 per-split chunk schedules ----
    perl2 = [[] for _ in range(K)]
    for c in range(CORES):
        s_, d_ = per_core[c]
        oc = owner_of[s_]
        lr = node2row[s_] - oc * NPCP
        gq = lr // 128
        for k in range(K):
            ink = (gq >= GS[k]) & (gq < GS[k + 1])
            perl2[k].append(((oc * HS[k] + lr - GS[k] * 128)[ink], d_[ink]))
    meta2 = []
    for k in range(K):
        cnt = np.zeros((CORES, NT), np.int64)
        for c in range(CORES):
            cnt[c] = np.bincount(perl2[k][c][1] // 128, minlength=NT)
        meta2.append(_chunk_schedule(cnt, CPT2, k == 0))

    in_maps = []
    shared = {}
    for nm, key in zip(["w1r", "w1n", "w2r", "w2n"],
                       ["W1_root", "W1_nbr", "W2_root", "W2_nbr"]):
        shared[nm] = np.asarray(inputs[key], np.float32).astype(NPBF)
    lin_w = np.asarray(inputs["lin_w"], np.float32)
    shared["la"] = np.ascontiguousarray(lin_w[:, :D].T).astype(NPBF)
    shared["lb"] = np.ascontiguousarray(lin_w[:, D:].T).astype(NPBF)
    shared["b1"] = np.asarray(inputs["b1"], np.float32).reshape(D, 1)
    shared["b2"] = np.asarray(inputs["b2"], np.float32).reshape(D, 1)
    shared["lbias"] = np.asarray(inputs["lin_b"], np.float32).reshape(D, 1)
    shared["zeros"] = np.zeros((NPCP, DP), NPBF)
    # constant layer-1 one-hots: ohc[s, j, d] = (d == DPC*j + s//BS)
    s_arr = np.arange(128)[:, None, None]
    j_arr = np.arange(128 // DPC)[None, :, None]
    d_arr = np.arange(128)[None, None, :]
    shared["ohc"] = (d_arr == DPC * j_arr + s_arr // BS).astype(NPBF)

    SENT = np.float32(500.0)  # one-hot sentinel: matches no iota column
    inv_deg = (1.0 / np.maximum(deg_in, 1)).astype(np.float32)

    for c in range(CORES):
        m = dict(shared)
        xT = np.zeros((D, NPCP), NPBF)
        mine = owner_of == c
        lrow = node2row[mine] - c * NPCP
        xT[:, lrow] = x_bf[mine].T
        m["xT"] = xT
        # 1/deg for this core's nodes, [p, g] layout (pad rows -> 1.0)
        invd = np.ones((128, NT), np.float32)
        invd[lrow % 128, lrow // 128] = inv_deg[mine]
        m["invd"] = invd

        s_, d_ = per_core[c]
        bounds = np.searchsorted(d_, np.arange(NT + 1) * 128)

        # layer-1 message stream in fixed-block layout: chunk with pattern
        # j holds dst rows DPC*j + s//BS; slot s serves edge BS*rep + s%BS
        # of that dst (zero rows pad short blocks)
        SLAB = cfg["SLAB"]
        nslab = -(-nt1 // SLAB)
        msg_lin = np.zeros((nslab * SLAB * CPT1 * 128, DP), NPBF)
        drow_bnd = np.searchsorted(d_, np.arange(NPCP + 1))
        for g in range(NT):
            reps_done = np.zeros(128 // DPC, np.int64)
            for ci in range(sl1[g]):
                j = pat1[st1[g] + ci]
                rep = reps_done[j]
                reps_done[j] += 1
                base = (st1[g] + ci) * 128
                for t in range(DPC):
                    drow = g * 128 + DPC * j + t
                    lo_, hi_ = drow_bnd[drow], drow_bnd[drow + 1]
                    e0 = lo_ + rep * BS
                    ne = min(BS, hi_ - e0)
                    if ne > 0:
                        sl_s = base + t * BS
                        msg_lin[sl_s:sl_s + ne, :D] = x_bf[s_[e0:e0 + ne]]
        m["msg1"] = np.ascontiguousarray(
            msg_lin.reshape(nslab, SLAB * CPT1, 128, DP)
            .transpose(0, 2, 1, 3).reshape(nslab, 128, SLAB * CPT1 * DP))

        # layer-2 gather indices + dst values, per table split
        for k in range(K):
            sv, dv = perl2[k][c]
            sl, st, nt, _ = meta2[k]
            L = nt * T2
            si = np.zeros(L, np.int64)
            dval = np.full(L, SENT, np.float32)
            bnd = np.searchsorted(dv, np.arange(NT + 1) * 128)
            for g in range(NT):
                lo_, hi_ = bnd[g], bnd[g + 1]
                kk = hi_ - lo_
                assert kk <= sl[g] * 128, (k, g, kk, sl[g])
                pos = st[g] * 128
                si[pos:pos + kk] = sv[lo_:hi_]
                dval[pos:pos + kk] = (dv[lo_:hi_] - g * 128).astype(np.float32)
            wi = _wrap_idxs(si, nt, T2)  # [nt, 128, S]
            m[f"si_2{k}"] = np.ascontiguousarray(
                wi.transpose(1, 0, 2).reshape(128, nt * (T2 // 16)))
            m[f"dv_2{k}"] = np.ascontiguousarray(
                dval.reshape(nt, CPT2, 128).transpose(2, 0, 1)
                .reshape(128, nt * CPT2)).astype(NPBF)
        in_maps.append(m)

    meta = dict(sl1=sl1, st1=st1, nt1=nt1, tcpt1=tcpt1, meta2=meta2,
                pat1=pat1)
    return in_maps, meta, node2row


def _build(cfg, meta):
    N, D, CORES, K = cfg["N"], cfg["D"], cfg["CORES"], cfg["K"]
    NPCP, NT, DP = cfg["NPCP"], cfg["NT"], cfg["DP"]
    T1, CPT1, T2, CPT2 = cfg["T1"], cfg["CPT1"], cfg["T2"], cfg["CPT2"]
    GS, HS = cfg["GS"], cfg["HS"]
    SLAB = cfg["SLAB"]
    S2 = T2 // 16
    sl1, st1, nt1, tcpt1 = (meta[k] for k in ("sl1", "st1", "nt1", "tcpt1"))
    meta2, pat1 = meta["meta2"], meta["pat1"]
    nslab = -(-nt1 // SLAB)

    NQ = 4  # SWDGE queues, round-robined so transfers overlap
    nc = bacc.Bacc("TRN2", target_bir_lowering=False, debug=False,
                   enable_asserts=True, num_devices=CORES,
                   num_swdge_queues=NQ)

    # --- I/O ---
    xT = nc.dram_tensor("xT", [D, NPCP], BF16, kind="ExternalInput")
    invd_in = nc.dram_tensor("invd", [128, NT], F32, kind="ExternalInput")
    w_in = {nm: nc.dram_tensor(nm, [D, D], BF16, kind="ExternalInput")
            for nm in ["w1r", "w1n", "w2r", "w2n", "la", "lb"]}
    b_in = {nm: nc.dram_tensor(nm, [D, 1], F32, kind="ExternalInput")
            for nm in ["b1", "b2", "lbias"]}
    zeros_in = nc.dram_tensor("zeros", [NPCP, DP], BF16, kind="ExternalInput")
    msg1_in = nc.dram_tensor("msg1", [nslab, 128, SLAB * CPT1 * DP], BF16,
                             kind="ExternalInput")
    ohc_in = nc.dram_tensor("ohc", [128, 8, 128], BF16, kind="ExternalInput")
    si_in, dv_in = [], []
    for k in range(K):
        nt_k = meta2[k][2]
        si_in.append(nc.dram_tensor(f"si_2{k}", [128, nt_k * S2], I16,
                                    kind="ExternalInput"))
        dv_in.append(nc.dram_tensor(f"dv_2{k}", [128, nt_k * CPT2], BF16,
                                    kind="ExternalInput"))
    out_T = nc.dram_tensor("out_T", [D, NPCP], F32, kind="ExternalOutput")

    # --- internal DRAM: h1 in K third-tables so the all-gathers pipeline ---
    h1own = [nc.dram_tensor(f"h1own{k}", [HS[k], DP], BF16) for k in range(K)]
    h1full = [nc.dram_tensor(f"h1full{k}", [CORES * HS[k], DP], BF16,
                             addr_space="Shared") for k in range(K)]

    with tile.TileContext(nc) as tc:
        with tc.tile_pool(name="const", bufs=1) as const, \
             tc.tile_pool(name="resident", bufs=1) as res, \
             tc.tile_pool(name="idx", bufs=1) as idxp, \
             tc.tile_pool(name="msg", bufs=8) as msgp, \
             tc.tile_pool(name="oh", bufs=8) as ohp, \
             tc.tile_pool(name="node", bufs=8) as nodep, \
             tc.tile_pool(name="ps_g", bufs=3, space="PSUM") as ps_g, \
             tc.tile_pool(name="ps_t", bufs=2, space="PSUM") as ps_t, \
             tc.tile_pool(name="ps_mm", bufs=2, space="PSUM") as ps_mm:

            ident_bf = const.tile([128, 128], BF16, tag="id_bf")
            make_identity(nc, ident_bf[:])
            ident_f = const.tile([128, 128], F32, tag="id_f")
            make_identity(nc, ident_f[:])
            # iota_bf[p, c, j] = j -- one-hot compare target (layer-2 tiles)
            iota_i = const.tile([128, CPT2, 128], I32)
            nc.gpsimd.iota(iota_i[:], pattern=[[0, CPT2], [1, 128]], base=0,
                           channel_multiplier=0)
            iota_bf = const.tile([128, CPT2, 128], BF16)
            nc.vector.tensor_copy(iota_bf[:], iota_i[:])
            ohc_sb = const.tile([128, 8, 128], BF16, tag="ohc")
            nc.sync.dma_start(ohc_sb[:], ohc_in[:, :, :])

            qctr = [0]

            w_sb = {}
            for nm, h in w_in.items():
                w_sb[nm] = const.tile([D, D], BF16, tag=f"w_{nm}",
                                      name=f"w_{nm}")
                nc.sync.dma_start(w_sb[nm][:], h[:, :])
            b_sb = {}
            for nm, h in b_in.items():
                b_sb[nm] = const.tile([D, 1], F32, tag=f"b_{nm}",
                                      name=f"b_{nm}")
                nc.sync.dma_start(b_sb[nm][:], h[:, :])
            invd_sb = const.tile([128, NT], F32, tag="invd")
            nc.sync.dma_start(invd_sb[:], invd_in[:, :])

            h1T_sb = res.tile([D, NPCP], BF16, tag="h1T")
            xT_sb = res.tile([D, NPCP], BF16, tag="xT_sb")
            nc.sync.dma_start(xT_sb[:], xT[:, :])

            # zero the h1 gather tables (pad cols must stay finite)
            for k in range(K):
                nc.sync.dma_start(h1own[k][:, :], zeros_in[0:HS[k], :])

            # layer-2 index/dst-value arrays, one DMA each
            idx_sb, dv2_sb = [], []
            for k in range(K):
                nt_k = meta2[k][2]
                t_ = idxp.tile([128, nt_k * S2], I16, tag=f"si2{k}",
                               name=f"si2{k}")
                nc.sync.dma_start(t_[:], si_in[k][:, :])
                idx_sb.append(t_)
                t_ = idxp.tile([128, nt_k * CPT2], BF16, tag=f"dv2{k}",
                               name=f"dv2{k}")
                nc.sync.dma_start(t_[:], dv_in[k][:, :])
                dv2_sb.append(t_)

            # ---------- layer 1: host-streamed messages ----------
            slabs = {}

            def get_slab1(sb):
                if sb not in slabs:
                    slab = msgp.tile([128, SLAB * CPT1 * DP], BF16,
                                     tag="msg1", name="msg1", bufs=4)
                    # big stream slabs ride the sync HWDGE ring; the
                    # scalar engine keeps only critical-chain activations
                    nc.sync.dma_start(slab[:], msg1_in[sb, :, :])
                    slabs[sb] = slab
                return slabs[sb]

            def l1_group(g):
                psg = ps_g.tile([128, 128], F32, tag="grp", name="grp")
                chunks = [st1[g] + j for j in range(sl1[g])]
                for ci, ch in enumerate(chunks):
                    ti, kk = divmod(int(ch), CPT1)
                    sb, sub = divmod(ti, SLAB)
                    slab = get_slab1(sb)
                    off = (sub * CPT1 + kk) * DP
                    # psum [node, feat] = const-onehot.T @ msg
                    nc.tensor.matmul(psg[:], ohc_sb[:, pat1[int(ch)], :],
                                     slab[:, off:off + DP],
                                     start=ci == 0, stop=ci == len(chunks) - 1)
                sl = slice(g * 128, (g + 1) * 128)
                a_nm = nodep.tile([128, 128], BF16, tag="a_nm", name="a_nm")
                nc.scalar.activation(a_nm[:], psg[:],
                                     mybir.ActivationFunctionType.Identity)
                a_ps = ps_t.tile([128, 128], BF16, tag="tr", name="tr")
                nc.tensor.transpose(a_ps[:D, :], a_nm[:, :D], ident_bf[:])
                aT_bf = nodep.tile([D, 128], BF16, tag="aT_bf", name="aT_bf")
                nc.vector.tensor_copy(aT_bf[:], a_ps[:D, :])
                hps = ps_mm.tile([D, 128], F32, tag="mm", name="mm")
                nc.tensor.matmul(hps[:], w_sb["w1r"][:], xT_sb[:, sl],
                                 start=True, stop=False)
                nc.tensor.matmul(hps[:], w_sb["w1n"][:], aT_bf[:],
                                 start=False, stop=True)
                nc.scalar.activation(h1T_sb[:, sl], hps[:],
                                     mybir.ActivationFunctionType.Identity,
                                     bias=b_sb["b1"][:, 0:1])
                h_ps = ps_t.tile([128, 128], BF16, tag="tr", name="tr")
                nc.tensor.transpose(h_ps[:, :D], h1T_sb[:, sl],
                                    ident_bf[:D, :D])
                h_nm = nodep.tile([128, D], BF16, tag="h_nm", name="h_nm")
                nc.vector.tensor_copy(h_nm[:], h_ps[:, :D])
                k = next(i for i in range(K) if g < GS[i + 1])
                slK = slice(g * 128 - GS[k] * 128, (g + 1) * 128 - GS[k] * 128)
                nc.sync.dma_start(h1own[k][slK, 0:D], h_nm[:])
                # fire each third-table's all-gather as soon as it is
                # complete, overlapping it with the rest of layer 1
                if g == GS[k + 1] - 1:
                    nc.gpsimd.collective_compute(
                        "AllGather", mybir.AluOpType.bypass,
                        replica_groups=[list(range(CORES))],
                        ins=[h1own[k].ap()], outs=[h1full[k].ap()],
                    )

            # ---------- layer 2: SWDGE gathers from the bf16 tables ----------
            state2 = [dict() for _ in range(K)]

            def get_tile2(k, ti):
                st = state2[k]
                if ti not in st:
                    cpt_t = meta2[k][3][ti]
                    nv = cpt_t * 128
                    si = idx_sb[k][:, ti * S2:ti * S2 + nv // 16]
                    dv = dv2_sb[k][:, ti * CPT2:ti * CPT2 + cpt_t]
                    msg = msgp.tile([128, CPT2, DP], BF16, tag="msg2",
                                    name="msg2", bufs=24)
                    q = qctr[0] % NQ
                    qctr[0] += 1
                    nc.gpsimd.dma_gather(msg[:, :cpt_t],
                                         h1full[k][0:CORES * HS[k], :], si,
                                         nv, nv, DP, elem_step=DP,
                                         queue_num=q)
                    oh = ohp.tile([128, CPT2, 128], BF16, tag="oh2",
                                  name="oh2")
                    nc.vector.tensor_tensor(
                        out=oh[:, :cpt_t], in0=iota_bf[:, :cpt_t],
                        in1=dv.to_broadcast([128, cpt_t, 128]),
                        op=mybir.AluOpType.is_equal)
                    st[ti] = (msg, oh)
                return st[ti]

            partial = res.tile([128, NT * 128], BF16, tag="partial")

            def part_post(g, psg):
                nc.scalar.activation(partial[:, g * 128:(g + 1) * 128],
                                     psg[:],
                                     mybir.ActivationFunctionType.Identity)

            def l2_post(g, psg):
                sl = slice(g * 128, (g + 1) * 128)
                a_bf = nodep.tile([128, DP], BF16, tag="a_bf", name="a_bf")
                nc.scalar.activation(a_bf[:], psg[:],
                                     mybir.ActivationFunctionType.Identity,
                                     scale=invd_sb[:, g:g + 1])
                a_ps = ps_t.tile([128, 128], BF16, tag="tr", name="tr")
                nc.tensor.transpose(a_ps[:], a_bf[:], ident_bf[:])
                aT = nodep.tile([D, 128], BF16, tag="aT2", name="aT2")
                nc.vector.tensor_copy(aT[:], a_ps[:D, :])

                hps = ps_mm.tile([D, 128], F32, tag="mm", name="mm")
                nc.tensor.matmul(hps[:], w_sb["w2r"][:], h1T_sb[:, sl],
                                 start=True, stop=False)
                nc.tensor.matmul(hps[:], w_sb["w2n"][:], aT[:],
                                 start=False, stop=True)
                h2T = nodep.tile([D, 128], BF16, tag="h2T", name="h2T")
                nc.scalar.activation(h2T[:], hps[:],
                                     mybir.ActivationFunctionType.Identity,
                                     bias=b_sb["b2"][:, 0:1])

                ops = ps_mm.tile([D, 128], F32, tag="mm_out", name="mm_out",
                                 bufs=1)
                nc.tensor.matmul(ops[:], w_sb["la"][:], h1T_sb[:, sl],
                                 start=True, stop=False)
                nc.tensor.matmul(ops[:], w_sb["lb"][:], h2T[:],
                                 start=False, stop=True)
                oT = nodep.tile([D, 128], F32, tag="oT", name="oT")
                nc.scalar.activation(oT[:], ops[:],
                                     mybir.ActivationFunctionType.Relu,
                                     bias=b_sb["lbias"][:, 0:1])
                nc.sync.dma_start(out_T[:, sl], oT[:])

            def l2_group(k, g, inject, post):
                sl_k, st_k = meta2[k][0], meta2[k][1]
                psg = ps_g.tile([128, 128], F32, tag="grp", name="grp")
                chunks = [st_k[g] + j for j in range(sl_k[g])]
                nch = len(chunks) + (1 if inject else 0)
                if inject:
                    # identity matmul accumulates the prior partial sum
                    nc.tensor.matmul(psg[:], ident_bf[:],
                                     partial[:, g * 128:(g + 1) * 128],
                                     start=True, stop=nch == 1)
                for ci, ch in enumerate(chunks):
                    ti, kk = divmod(int(ch), CPT2)
                    msg, oh = get_tile2(k, ti)
                    first = ci == 0 and not inject
                    last = ci == len(chunks) - 1
                    # psum [node, feat] = onehot.T @ msg
                    nc.tensor.matmul(psg[:], oh[:, kk, :], msg[:, kk, :],
                                     start=first, stop=last)
                post(g, psg)

            # ---------- emission schedule ----------
            # layer 1 in full (its all-gathers fire per third), then the K
            # layer-2 passes; deep msg2 buffering lets pass-0 gathers run
            # ~16 tiles ahead while the PE drains the tail of layer 1
            for g in range(NT):
                l1_group(g)
            for k in range(K):
                post = l2_post if k == K - 1 else part_post
                for g in range(NT):
                    l2_group(k, g, k > 0, post)

    nc.compile()
    return nc


def build_and_run(inputs, cfg=None, trace=False, **run_kwargs):
    cfg = _derive(cfg or DEFAULT_CFG)
    in_maps, meta, node2row = _prep(inputs, cfg)
    nc = _build(cfg, meta)
    res = run_bass_kernel_spmd(nc, in_maps, list(range(cfg["CORES"])),
                               trace=trace, **run_kwargs)
    N, NPCP, D = cfg["N"], cfg["NPCP"], cfg["D"]
    out = np.empty((N, D), np.float32)
    owner_of = node2row // NPCP
    local = node2row - owner_of * NPCP
    for c in range(cfg["CORES"]):
        mine = owner_of == c
        out[mine] = res.results[c]["out_T"][:, local[mine]].T
    return out, res


def kernel(**inputs) -> np.ndarray:
    out, _ = build_and_run(inputs)
    return out


# revision 61
# speedup vs baseline: 1.0137x; 1.0137x over previous
"""Trainium2 Bass kernel for a 2-layer GNN message-passing block (SAGE-style).

Computation (see reference):
    h1 = x @ W1_root + seg_sum(x[src], dst) @ W1_nbr + b1
    a2 = seg_sum(h1[src], dst) / max(deg, 1)
    h2 = h1 @ W2_root + a2 @ W2_nbr + b2
    out = relu(h1 @ lin_w[:, :D].T + h2 @ lin_w[:, D:].T + lin_b)

Sharding: nodes are dealt to (core, 128-node group) slots in descending
in-degree order (snake), which balances per-group edge counts across cores;
edges are sharded by destination so the segment reduction is device-local.

Profiling-driven design notes (the original kernel spent ~73% of its span
on SWDGE descriptor generation; the sustained dma_gather floor is ~2.5us
per 1024-descriptor instruction, DMA-round-trip bound with 4 queues):
  * Layer-1 messages x[src] are a pure gather of the *input*, so they are
    materialized host-side as a dst-sorted bf16 stream (padded to 128
    cols so LDWEIGHTS gets fast-weight-load) and read with sequential
    DMA in 4-tile slabs — no SWDGE descriptors for layer 1.
  * Everything on the PE runs in bf16 (f32 PSUM accumulation); gather
    tables are bf16 so layer-2 SWDGE gathers move 256B/edge.
  * Layer-2 keeps the SWDGE dma_gather path (h1 is device-computed): 4
    queues round-robin, edges host-sorted by destination, segment sum via
    one-hot matmuls into one PSUM tile per node group (dma_scatter_add
    loses colliding row updates on HW, so no scatter is used). 1/deg is
    a pure function of edge_index, so it is computed host-side.
  * Layer-1's one-hots are *constant*: nodes are dealt round-robin by
    in-degree rank, so rows within a group are degree-sorted and the
    host packs each dst's edges into fixed 8-slot blocks (16 dsts per
    chunk). One constant pattern tile replaces all per-tile DVE
    is_equal builds, which previously paced layer 1.
  * h1 lives in two tables (30/19 groups; the int16 gather-index limit
    caps a table at 31 groups x 8 cores). Each AllGather fires as soon
    as its table is written; the asymmetric split makes pass 0 long
    enough to hide the second AllGather. Layer 2 runs as K=2 passes;
    pass-0 partial sums park in SBUF (bf16) and are injected back into
    PSUM via an identity matmul during pass 1. Deep (24-buffer) msg
    tiles let gathers run well ahead of their consuming matmuls.
  * All index/dst-value arrays are uploaded in partition-major layouts
    and loaded with one big DMA each.

Dense math runs feature-major: weights load as stationary lhsT once and
node columns stream as rhs. The final output is produced transposed and
scattered back to original node order on the host.
"""
import sys

sys.path.insert(0, "/opt/trn_rl_repo")

import numpy as np
import ml_dtypes

import concourse.bass as bass
import concourse.mybir as mybir
from concourse import bacc, tile
from concourse.bass_utils import run_bass_kernel_spmd
from concourse.masks import make_identity

F32 = mybir.dt.float32
BF16 = mybir.dt.bfloat16
I16 = mybir.dt.int16
I32 = mybir.dt.int32
NPBF = ml_dtypes.bfloat16

DEFAULT_CFG = dict(
    N=50000,      # nodes
    D=96,         # feature dim
    CORES=8,
    T1=2048,      # edge slots per layer-1 stream tile
    T2=1024,      # edge slots per layer-2 gather tile (ucode ring: <=1024)
    SLAB=4,       # layer-1 stream tiles per DMA (bigger descriptors)
    K=2,          # h1 table splits (pipelined all-gathers / layer-2 passes)
    SPLIT0=30,    # groups in table 0 (pass-0 share of layer-2 edges)
)


def _derive(cfg):
    c = dict(cfg)
    c["NPC"] = c["N"] // c["CORES"]              # nodes per core (logical)
    c["NPCP"] = -(-c["NPC"] // 128) * 128        # padded to node groups
    c["NT"] = c["NPCP"] // 128                   # node groups per core
    c["DP"] = 128                                # padded feature dim
    c["CPT1"] = c["T1"] // 128                   # chunks per layer-1 tile
    c["CPT2"] = c["T2"] // 128                   # chunks per layer-2 tile
    K = c["K"]
    if K == 2 and c.get("SPLIT0"):
        nts = [c["SPLIT0"], c["NT"] - c["SPLIT0"]]
    else:
        base, rem = divmod(c["NT"], K)
        nts = [base + (1 if k < rem else 0) for k in range(K)]
    c["GS"] = np.concatenate([[0], np.cumsum(nts)]).tolist()  # group bounds
    c["HS"] = [n * 128 for n in nts]                          # rows per core
    for h in c["HS"]:
        assert c["CORES"] * h < 32768, "int16 gather index overflow"
    return c


def _wrap_idxs(arr, n_tiles, T):
    """int arr [n_tiles*T] -> [n_tiles, 128, T//16] int16 in the SWDGE
    wrapped layout: element (t, p, s) = arr[t*T + s*16 + p%16]."""
    w = arr.reshape(n_tiles, T // 16, 16).transpose(0, 2, 1)  # [nt, 16, S]
    return np.ascontiguousarray(np.tile(w, (1, 8, 1)).astype(np.int16))


def _chunk_schedule(cnt_by_core, CPT, floor1):
    """cnt_by_core [CORES, NT] -> uniform-across-cores chunk schedule."""
    sl = (-(-cnt_by_core // 128)).max(axis=0)
    if floor1:
        # every group needs >=1 chunk so its PSUM tile is always written
        sl = np.maximum(sl, 1)
    starts = np.concatenate([[0], np.cumsum(sl)])
    tot = max(1, int(sl.sum()))
    n_tiles = -(-tot // CPT)
    tile_cpt = [min(CPT, max(1, tot - t * CPT)) for t in range(n_tiles)]
    return sl, starts, n_tiles, tile_cpt


def _prep(inputs, cfg):
    """Host-side sharding. Returns (in_maps, meta, node2row) where
    node2row[n] is the node's row in the padded per-core layout."""
    N, D, CORES, K = cfg["N"], cfg["D"], cfg["CORES"], cfg["K"]
    NPCP, NT, DP = cfg["NPCP"], cfg["NT"], cfg["DP"]
    T1, CPT1, T2, CPT2 = cfg["T1"], cfg["CPT1"], cfg["T2"], cfg["CPT2"]
    GS, HS = cfg["GS"], cfg["HS"]

    x = np.asarray(inputs["x"], np.float32)
    x_bf = x.astype(NPBF)
    ei = np.asarray(inputs["edge_index"]).astype(np.int64)
    src, dst = ei[0], ei[1]

    # deal nodes round-robin by in-degree rank: rank r -> core r%CORES,
    # per-core slot r//CORES (group-major). Every core sees the same degree
    # sequence (edge balance), and rows within a group are degree-sorted so
    # fixed-block layer-1 chunks waste little padding.
    deg_in = np.bincount(dst, minlength=N)
    order_nodes = np.argsort(-deg_in, kind="stable")
    rank = np.empty(N, np.int64)
    rank[order_nodes] = np.arange(N)
    owner_of = rank % CORES
    q_ = rank // CORES
    assert q_.max() < NPCP
    node2row = owner_of * NPCP + q_  # global padded row

    owner = owner_of[dst]
    row_d = node2row[dst]

    # per-core dst-sorted edge lists
    per_core = []
    for c in range(CORES):
        sel = owner == c
        d = row_d[sel] - c * NPCP
        order = np.argsort(d, kind="stable")
        per_core.append((src[sel][order], d[order]))

    # ---- layer-1 fixed-block schedule ----
    # chunk pattern j covers dsts 16j..16j+15 of its group, 8 slots each;
    # slice j repeats ceil(max_deg/8) times (uniform across cores because
    # the degree deal aligns slice degrees). One constant one-hot per j.
    BS, DPC = 8, 16            # slots per dst block, dsts per chunk
    # max degree per (group, slice) over all cores
    deg_row = np.zeros(CORES * NPCP, np.int64)
    deg_row[node2row] = deg_in
    deg_row = deg_row.reshape(CORES, NT, 128 // DPC, DPC)  # [c, g, slice, dst]
    nrep = np.maximum(-(-deg_row.max(axis=(0, 3)) // BS), 1)  # [NT, 8 slices]
    pat1, grp_chunks = [], []   # pattern id per global chunk, chunks per group
    for g in range(NT):
        cl = []
        for j in range(128 // DPC):
            cl += [j] * int(nrep[g, j])
        grp_chunks.append(len(cl))
        pat1 += cl
    sl1 = np.array(grp_chunks, np.int64)
    st1 = np.concatenate([[0], np.cumsum(sl1)])
    tot1 = int(sl1.sum())
    nt1 = -(-tot1 // CPT1)
    tcpt1 = [min(CPT1, max(1, tot1 - t * CPT1)) for t in range(nt1)]

    # ---- layer-2 per-split chunk schedules ----
    perl2 = [[] for _ in range(K)]
    for c in range(CORES):
        s_, d_ = per_core[c]
        oc = owner_of[s_]
        lr = node2row[s_] - oc * NPCP
        gq = lr // 128
        for k in range(K):
            ink = (gq >= GS[k]) & (gq < GS[k + 1])
            perl2[k].append(((oc * HS[k] + lr - GS[k] * 128)[ink], d_[ink]))
    meta2 = []
    for k in range(K):
        cnt = np.zeros((CORES, NT), np.int64)
        for c in range(CORES):
            cnt[c] = np.bincount(perl2[k][c][1] // 128, minlength=NT)
        meta2.append(_chunk_schedule(cnt, CPT2, k == 0))

    in_maps = []
    shared = {}
    for nm, key in zip(["w1r", "w1n", "w2r", "w2n"],
                       ["W1_root", "W1_nbr", "W2_root", "W2_nbr"]):
        shared[nm] = np.asarray(inputs[key], np.float32).astype(NPBF)
    lin_w = np.asarray(inputs["lin_w"], np.float32)
    shared["la"] = np.ascontiguousarray(lin_w[:, :D].T).astype(NPBF)
    shared["lb"] = np.ascontiguousarray(lin_w[:, D:].T).astype(NPBF)
    shared["b1"] = np.asarray(inputs["b1"], np.float32).reshape(D, 1)
    shared["b1r"] = np.asarray(inputs["b1"], np.float32).reshape(1, D).astype(NPBF)
    shared["b2"] = np.asarray(inputs["b2"], np.float32).reshape(D, 1)
    shared["lbias"] = np.asarray(inputs["lin_b"], np.float32).reshape(D, 1)
    shared["zeros"] = np.zeros((NPCP, DP), NPBF)
    # constant layer-1 one-hots: ohc[s, j, d] = (d == DPC*j + s//BS)
    s_arr = np.arange(128)[:, None, None]
    j_arr = np.arange(128 // DPC)[None, :, None]
    d_arr = np.arange(128)[None, None, :]
    shared["ohc"] = (d_arr == DPC * j_arr + s_arr // BS).astype(NPBF)

    SENT = np.float32(500.0)  # one-hot sentinel: matches no iota column
    inv_deg = (1.0 / np.maximum(deg_in, 1)).astype(np.float32)

    for c in range(CORES):
        m = dict(shared)
        xT = np.zeros((D, NPCP), NPBF)
        mine = owner_of == c
        lrow = node2row[mine] - c * NPCP
        xT[:, lrow] = x_bf[mine].T
        m["xT"] = xT
        # 1/deg for this core's nodes, [p, g] layout (pad rows -> 1.0)
        invd = np.ones((128, NT), np.float32)
        invd[lrow % 128, lrow // 128] = inv_deg[mine]
        m["invd"] = invd

        s_, d_ = per_core[c]
        bounds = np.searchsorted(d_, np.arange(NT + 1) * 128)

        # layer-1 message stream in fixed-block layout: chunk with pattern
        # j holds dst rows DPC*j + s//BS; slot s serves edge BS*rep + s%BS
        # of that dst (zero rows pad short blocks)
        SLAB = cfg["SLAB"]
        nslab = -(-nt1 // SLAB)
        msg_lin = np.zeros((nslab * SLAB * CPT1 * 128, DP), NPBF)
        drow_bnd = np.searchsorted(d_, np.arange(NPCP + 1))
        for g in range(NT):
            reps_done = np.zeros(128 // DPC, np.int64)
            for ci in range(sl1[g]):
                j = pat1[st1[g] + ci]
                rep = reps_done[j]
                reps_done[j] += 1
                base = (st1[g] + ci) * 128
                for t in range(DPC):
                    drow = g * 128 + DPC * j + t
                    lo_, hi_ = drow_bnd[drow], drow_bnd[drow + 1]
                    e0 = lo_ + rep * BS
                    ne = min(BS, hi_ - e0)
                    if ne > 0:
                        sl_s = base + t * BS
                        msg_lin[sl_s:sl_s + ne, :D] = x_bf[s_[e0:e0 + ne]]
        m["msg1"] = np.ascontiguousarray(
            msg_lin.reshape(nslab, SLAB * CPT1, 128, DP)
            .transpose(0, 2, 1, 3).reshape(nslab, 128, SLAB * CPT1 * DP))

        # layer-2 gather indices + dst values, per table split
        for k in range(K):
            sv, dv = perl2[k][c]
            sl, st, nt, _ = meta2[k]
            L = nt * T2
            si = np.zeros(L, np.int64)
            dval = np.full(L, SENT, np.float32)
            bnd = np.searchsorted(dv, np.arange(NT + 1) * 128)
            for g in range(NT):
                lo_, hi_ = bnd[g], bnd[g + 1]
                kk = hi_ - lo_
                assert kk <= sl[g] * 128, (k, g, kk, sl[g])
                pos = st[g] * 128
                si[pos:pos + kk] = sv[lo_:hi_]
                dval[pos:pos + kk] = (dv[lo_:hi_] - g * 128).astype(np.float32)
            wi = _wrap_idxs(si, nt, T2)  # [nt, 128, S]
            m[f"si_2{k}"] = np.ascontiguousarray(
                wi.transpose(1, 0, 2).reshape(128, nt * (T2 // 16)))
            m[f"dv_2{k}"] = np.ascontiguousarray(
                dval.reshape(nt, CPT2, 128).transpose(2, 0, 1)
                .reshape(128, nt * CPT2)).astype(NPBF)
        in_maps.append(m)

    meta = dict(sl1=sl1, st1=st1, nt1=nt1, tcpt1=tcpt1, meta2=meta2,
                pat1=pat1)
    return in_maps, meta, node2row


def _build(cfg, meta):
    N, D, CORES, K = cfg["N"], cfg["D"], cfg["CORES"], cfg["K"]
    NPCP, NT, DP = cfg["NPCP"], cfg["NT"], cfg["DP"]
    T1, CPT1, T2, CPT2 = cfg["T1"], cfg["CPT1"], cfg["T2"], cfg["CPT2"]
    GS, HS = cfg["GS"], cfg["HS"]
    SLAB = cfg["SLAB"]
    S2 = T2 // 16
    sl1, st1, nt1, tcpt1 = (meta[k] for k in ("sl1", "st1", "nt1", "tcpt1"))
    meta2, pat1 = meta["meta2"], meta["pat1"]
    nslab = -(-nt1 // SLAB)

    NQ = 4  # SWDGE queues, round-robined so transfers overlap
    nc = bacc.Bacc("TRN2", target_bir_lowering=False, debug=False,
                   enable_asserts=True, num_devices=CORES,
                   num_swdge_queues=NQ)

    # --- I/O ---
    xT = nc.dram_tensor("xT", [D, NPCP], BF16, kind="ExternalInput")
    invd_in = nc.dram_tensor("invd", [128, NT], F32, kind="ExternalInput")
    w_in = {nm: nc.dram_tensor(nm, [D, D], BF16, kind="ExternalInput")
            for nm in ["w1r", "w1n", "w2r", "w2n", "la", "lb"]}
    b_in = {nm: nc.dram_tensor(nm, [D, 1], F32, kind="ExternalInput")
            for nm in ["b1", "b2", "lbias"]}
    zeros_in = nc.dram_tensor("zeros", [NPCP, DP], BF16, kind="ExternalInput")
    b1r_in = nc.dram_tensor("b1r", [1, D], BF16, kind="ExternalInput")
    msg1_in = nc.dram_tensor("msg1", [nslab, 128, SLAB * CPT1 * DP], BF16,
                             kind="ExternalInput")
    ohc_in = nc.dram_tensor("ohc", [128, 8, 128], BF16, kind="ExternalInput")
    si_in, dv_in = [], []
    for k in range(K):
        nt_k = meta2[k][2]
        si_in.append(nc.dram_tensor(f"si_2{k}", [128, nt_k * S2], I16,
                                    kind="ExternalInput"))
        dv_in.append(nc.dram_tensor(f"dv_2{k}", [128, nt_k * CPT2], BF16,
                                    kind="ExternalInput"))
    out_T = nc.dram_tensor("out_T", [D, NPCP], F32, kind="ExternalOutput")

    # --- internal DRAM: h1 in K third-tables so the all-gathers pipeline ---
    h1own = [nc.dram_tensor(f"h1own{k}", [HS[k], DP], BF16) for k in range(K)]
    h1full = [nc.dram_tensor(f"h1full{k}", [CORES * HS[k], DP], BF16,
                             addr_space="Shared") for k in range(K)]

    with tile.TileContext(nc) as tc:
        with tc.tile_pool(name="const", bufs=1) as const, \
             tc.tile_pool(name="resident", bufs=1) as res, \
             tc.tile_pool(name="idx", bufs=1) as idxp, \
             tc.tile_pool(name="msg", bufs=8) as msgp, \
             tc.tile_pool(name="oh", bufs=8) as ohp, \
             tc.tile_pool(name="node", bufs=8) as nodep, \
             tc.tile_pool(name="ps_g", bufs=3, space="PSUM") as ps_g, \
             tc.tile_pool(name="ps_t", bufs=2, space="PSUM") as ps_t, \
             tc.tile_pool(name="ps_mm", bufs=2, space="PSUM") as ps_mm:

            ident_bf = const.tile([128, 128], BF16, tag="id_bf")
            make_identity(nc, ident_bf[:])
            ident_f = const.tile([128, 128], F32, tag="id_f")
            make_identity(nc, ident_f[:])
            # iota_bf[p, c, j] = j -- one-hot compare target (layer-2 tiles)
            iota_i = const.tile([128, CPT2, 128], I32)
            nc.gpsimd.iota(iota_i[:], pattern=[[0, CPT2], [1, 128]], base=0,
                           channel_multiplier=0)
            iota_bf = const.tile([128, CPT2, 128], BF16)
            nc.vector.tensor_copy(iota_bf[:], iota_i[:])
            ohc_sb = const.tile([128, 8, 128], BF16, tag="ohc")
            nc.sync.dma_start(ohc_sb[:], ohc_in[:, :, :])

            qctr = [0]

            w_sb = {}
            for nm, h in w_in.items():
                w_sb[nm] = const.tile([D, D], BF16, tag=f"w_{nm}",
                                      name=f"w_{nm}")
                nc.sync.dma_start(w_sb[nm][:], h[:, :])
            b_sb = {}
            for nm, h in b_in.items():
                b_sb[nm] = const.tile([D, 1], F32, tag=f"b_{nm}",
                                      name=f"b_{nm}")
                nc.sync.dma_start(b_sb[nm][:], h[:, :])
            invd_sb = const.tile([128, NT], F32, tag="invd")
            nc.sync.dma_start(invd_sb[:], invd_in[:, :])
            b1r_sb = const.tile([1, D], BF16, tag="b1r")
            nc.sync.dma_start(b1r_sb[:], b1r_in[:, :])
            ones_sb = const.tile([1, 128], BF16, tag="ones")
            nc.vector.memset(ones_sb[:], 1.0)

            h1T_sb = res.tile([D, NPCP], BF16, tag="h1T")
            xT_sb = res.tile([D, NPCP], BF16, tag="xT_sb")
            nc.sync.dma_start(xT_sb[:], xT[:, :])

            # zero the h1 gather tables (pad cols must stay finite)
            for k in range(K):
                nc.sync.dma_start(h1own[k][:, :], zeros_in[0:HS[k], :])

            # layer-2 index/dst-value arrays, one DMA each
            idx_sb, dv2_sb = [], []
            for k in range(K):
                nt_k = meta2[k][2]
                t_ = idxp.tile([128, nt_k * S2], I16, tag=f"si2{k}",
                               name=f"si2{k}")
                nc.sync.dma_start(t_[:], si_in[k][:, :])
                idx_sb.append(t_)
                t_ = idxp.tile([128, nt_k * CPT2], BF16, tag=f"dv2{k}",
                               name=f"dv2{k}")
                nc.sync.dma_start(t_[:], dv_in[k][:, :])
                dv2_sb.append(t_)

            # ---------- layer 1: host-streamed messages ----------
            slabs = {}

            def get_slab1(sb):
                if sb not in slabs:
                    slab = msgp.tile([128, SLAB * CPT1 * DP], BF16,
                                     tag="msg1", name="msg1", bufs=4)
                    # big stream slabs ride the sync HWDGE ring; the
                    # scalar engine keeps only critical-chain activations
                    nc.sync.dma_start(slab[:], msg1_in[sb, :, :])
                    slabs[sb] = slab
                return slabs[sb]

            def l1_group(g):
                psg = ps_g.tile([128, 128], F32, tag="grp", name="grp")
                chunks = [st1[g] + j for j in range(sl1[g])]
                for ci, ch in enumerate(chunks):
                    ti, kk = divmod(int(ch), CPT1)
                    sb, sub = divmod(ti, SLAB)
                    slab = get_slab1(sb)
                    off = (sub * CPT1 + kk) * DP
                    # psum [feat, node] = msg.T @ const-onehot (feature-major
                    # agg feeds both dense paths with no PE transpose)
                    nc.tensor.matmul(psg[:], slab[:, off:off + DP],
                                     ohc_sb[:, pat1[int(ch)], :],
                                     start=ci == 0, stop=ci == len(chunks) - 1)
                sl = slice(g * 128, (g + 1) * 128)
                aT_bf = nodep.tile([D, 128], BF16, tag="aT_bf", name="aT_bf")
                nc.scalar.activation(aT_bf[:], psg[:D, :],
                                     mybir.ActivationFunctionType.Identity)
                # node-major h1 for the gather table: the critical chain to
                # the all-gather is psum -> aT_bf -> 3 matmuls -> h_nm -> DMA
                hn = ps_t.tile([128, 128], F32, tag="tr", name="tr")
                nc.tensor.matmul(hn[:, :D], xT_sb[:, sl], w_sb["w1r"][:],
                                 start=True, stop=False)
                nc.tensor.matmul(hn[:, :D], aT_bf[:], w_sb["w1n"][:],
                                 start=False, stop=False)
                nc.tensor.matmul(hn[:, :D], ones_sb[:, :], b1r_sb[:, :],
                                 start=False, stop=True)
                h_nm = nodep.tile([128, D], BF16, tag="h_nm", name="h_nm")
                nc.scalar.activation(h_nm[:], hn[:, :D],
                                     mybir.ActivationFunctionType.Identity)
                # feature-major h1 (layer-2 dense + head), off the chain
                hps = ps_mm.tile([D, 128], F32, tag="mm", name="mm")
                nc.tensor.matmul(hps[:], w_sb["w1r"][:], xT_sb[:, sl],
                                 start=True, stop=False)
                nc.tensor.matmul(hps[:], w_sb["w1n"][:], aT_bf[:],
                                 start=False, stop=True)
                nc.scalar.activation(h1T_sb[:, sl], hps[:],
                                     mybir.ActivationFunctionType.Identity,
                                     bias=b_sb["b1"][:, 0:1])
                k = next(i for i in range(K) if g < GS[i + 1])
                slK = slice(g * 128 - GS[k] * 128, (g + 1) * 128 - GS[k] * 128)
                nc.sync.dma_start(h1own[k][slK, 0:D], h_nm[:])
                # fire each third-table's all-gather as soon as it is
                # complete, overlapping it with the rest of layer 1
                if g == GS[k + 1] - 1:
                    nc.gpsimd.collective_compute(
                        "AllGather", mybir.AluOpType.bypass,
                        replica_groups=[list(range(CORES))],
                        ins=[h1own[k].ap()], outs=[h1full[k].ap()],
                    )

            # ---------- layer 2: SWDGE gathers from the bf16 tables ----------
            state2 = [dict() for _ in range(K)]

            def get_tile2(k, ti):
                st = state2[k]
                if ti not in st:
                    cpt_t = meta2[k][3][ti]
                    nv = cpt_t * 128
                    si = idx_sb[k][:, ti * S2:ti * S2 + nv // 16]
                    dv = dv2_sb[k][:, ti * CPT2:ti * CPT2 + cpt_t]
                    msg = msgp.tile([128, CPT2, DP], BF16, tag="msg2",
                                    name="msg2", bufs=24)
                    q = qctr[0] % NQ
                    qctr[0] += 1
                    nc.gpsimd.dma_gather(msg[:, :cpt_t],
                                         h1full[k][0:CORES * HS[k], :], si,
                                         nv, nv, DP, elem_step=DP,
                                         queue_num=q)
                    oh = ohp.tile([128, CPT2, 128], BF16, tag="oh2",
                                  name="oh2")
                    nc.vector.tensor_tensor(
                        out=oh[:, :cpt_t], in0=iota_bf[:, :cpt_t],
                        in1=dv.to_broadcast([128, cpt_t, 128]),
                        op=mybir.AluOpType.is_equal)
                    st[ti] = (msg, oh)
                return st[ti]

            partial = res.tile([128, NT * 128], BF16, tag="partial")

            def part_post(g, psg):
                nc.scalar.activation(partial[:, g * 128:(g + 1) * 128],
                                     psg[:],
                                     mybir.ActivationFunctionType.Identity)

            def l2_post(g, psg):
                sl = slice(g * 128, (g + 1) * 128)
                a_bf = nodep.tile([128, DP], BF16, tag="a_bf", name="a_bf")
                nc.scalar.activation(a_bf[:], psg[:],
                                     mybir.ActivationFunctionType.Identity,
                                     scale=invd_sb[:, g:g + 1])
                a_ps = ps_t.tile([128, 128], BF16, tag="tr", name="tr")
                nc.tensor.transpose(a_ps[:], a_bf[:], ident_bf[:])
                aT = nodep.tile([D, 128], BF16, tag="aT2", name="aT2")
                nc.vector.tensor_copy(aT[:], a_ps[:D, :])

                hps = ps_mm.tile([D, 128], F32, tag="mm", name="mm")
                nc.tensor.matmul(hps[:], w_sb["w2r"][:], h1T_sb[:, sl],
                                 start=True, stop=False)
                nc.tensor.matmul(hps[:], w_sb["w2n"][:], aT[:],
                                 start=False, stop=True)
                h2T = nodep.tile([D, 128], BF16, tag="h2T", name="h2T")
                nc.scalar.activation(h2T[:], hps[:],
                                     mybir.ActivationFunctionType.Identity,
                                     bias=b_sb["b2"][:, 0:1])

                ops = ps_mm.tile([D, 128], F32, tag="mm_out", name="mm_out",
                                 bufs=1)
                nc.tensor.matmul(ops[:], w_sb["la"][:], h1T_sb[:, sl],
                                 start=True, stop=False)
                nc.tensor.matmul(ops[:], w_sb["lb"][:], h2T[:],
                                 start=False, stop=True)
                oT = nodep.tile([D, 128], F32, tag="oT", name="oT")
                nc.scalar.activation(oT[:], ops[:],
                                     mybir.ActivationFunctionType.Relu,
                                     bias=b_sb["lbias"][:, 0:1])
                nc.sync.dma_start(out_T[:, sl], oT[:])

            def l2_group(k, g, inject, post):
                sl_k, st_k = meta2[k][0], meta2[k][1]
                psg = ps_g.tile([128, 128], F32, tag="grp", name="grp")
                chunks = [st_k[g] + j for j in range(sl_k[g])]
                nch = len(chunks) + (1 if inject else 0)
                if inject:
                    # identity matmul accumulates the prior partial sum
                    nc.tensor.matmul(psg[:], ident_bf[:],
                                     partial[:, g * 128:(g + 1) * 128],
                                     start=True, stop=nch == 1)
                for ci, ch in enumerate(chunks):
                    ti, kk = divmod(int(ch), CPT2)
                    msg, oh = get_tile2(k, ti)
                    first = ci == 0 and not inject
                    last = ci == len(chunks) - 1
                    # psum [node, feat] = onehot.T @ msg
                    nc.tensor.matmul(psg[:], oh[:, kk, :], msg[:, kk, :],
                                     start=first, stop=last)
                post(g, psg)

            # ---------- emission schedule ----------
            # layer 1 in full (its all-gathers fire per third), then the K
            # layer-2 passes; deep msg2 buffering lets pass-0 gathers run
            # ~16 tiles ahead while the PE drains the tail of layer 1
            for g in range(NT):
                l1_group(g)
            for k in range(K):
                post = l2_post if k == K - 1 else part_post
                for g in range(NT):
                    l2_group(k, g, k > 0, post)

    nc.compile()
    return nc


def build_and_run(inputs, cfg=None, trace=False, **run_kwargs):
    cfg = _derive(cfg or DEFAULT_CFG)
    in_maps, meta, node2row = _prep(inputs, cfg)
    nc = _build(cfg, meta)
    res = run_bass_kernel_spmd(nc, in_maps, list(range(cfg["CORES"])),
                               trace=trace, **run_kwargs)
    N, NPCP, D = cfg["N"], cfg["NPCP"], cfg["D"]
    out = np.empty((N, D), np.float32)
    owner_of = node2row // NPCP
    local = node2row - owner_of * NPCP
    for c in range(cfg["CORES"]):
        mine = owner_of == c
        out[mine] = res.results[c]["out_T"][:, local[mine]].T
    return out, res


def kernel(**inputs) -> np.ndarray:
    out, _ = build_and_run(inputs)
    return out


# revision 64
# speedup vs baseline: 1.0398x; 1.0258x over previous
"""Trainium2 Bass kernel for a 2-layer GNN message-passing block (SAGE-style).

Computation (see reference):
    h1 = x @ W1_root + seg_sum(x[src], dst) @ W1_nbr + b1
    a2 = seg_sum(h1[src], dst) / max(deg, 1)
    h2 = h1 @ W2_root + a2 @ W2_nbr + b2
    out = relu(h1 @ lin_w[:, :D].T + h2 @ lin_w[:, D:].T + lin_b)

Sharding: nodes are dealt to (core, 128-node group) slots in descending
in-degree order (snake), which balances per-group edge counts across cores;
edges are sharded by destination so the segment reduction is device-local.

Profiling-driven design notes (the original kernel spent ~73% of its span
on SWDGE descriptor generation; the sustained dma_gather floor is ~2.5us
per 1024-descriptor instruction, DMA-round-trip bound with 4 queues):
  * Layer-1 messages x[src] are a pure gather of the *input*, so they are
    materialized host-side as a dst-sorted bf16 stream (padded to 128
    cols so LDWEIGHTS gets fast-weight-load) and read with sequential
    DMA in 4-tile slabs — no SWDGE descriptors for layer 1.
  * Everything on the PE runs in bf16 (f32 PSUM accumulation); gather
    tables are bf16 so layer-2 SWDGE gathers move 256B/edge.
  * Layer-2 keeps the SWDGE dma_gather path (h1 is device-computed): 4
    queues round-robin, edges host-sorted by destination, segment sum via
    one-hot matmuls into one PSUM tile per node group (dma_scatter_add
    loses colliding row updates on HW, so no scatter is used). 1/deg is
    a pure function of edge_index, so it is computed host-side.
  * Layer-1's one-hots are *constant*: nodes are dealt round-robin by
    in-degree rank, so rows within a group are degree-sorted and the
    host packs each dst's edges into fixed 8-slot blocks (16 dsts per
    chunk). One constant pattern tile replaces all per-tile DVE
    is_equal builds, which previously paced layer 1.
  * h1 lives in two tables (30/19 groups; the int16 gather-index limit
    caps a table at 31 groups x 8 cores). Each AllGather fires as soon
    as its table is written; the asymmetric split makes pass 0 long
    enough to hide the second AllGather. Layer 2 runs as K=2 passes;
    pass-0 partial sums park in SBUF (bf16) and are injected back into
    PSUM via an identity matmul during pass 1. Deep (24-buffer) msg
    tiles let gathers run well ahead of their consuming matmuls.
  * All index/dst-value arrays are uploaded in partition-major layouts
    and loaded with one big DMA each.

Dense math runs feature-major: weights load as stationary lhsT once and
node columns stream as rhs. The final output is produced transposed and
scattered back to original node order on the host.
"""
import sys

sys.path.insert(0, "/opt/trn_rl_repo")

import numpy as np
import ml_dtypes

import concourse.bass as bass
import concourse.mybir as mybir
from concourse import bacc, tile
from concourse.bass_utils import run_bass_kernel_spmd
from concourse.masks import make_identity

F32 = mybir.dt.float32
BF16 = mybir.dt.bfloat16
I16 = mybir.dt.int16
I32 = mybir.dt.int32
NPBF = ml_dtypes.bfloat16

DEFAULT_CFG = dict(
    N=50000,      # nodes
    D=96,         # feature dim
    CORES=8,
    T1=2048,      # edge slots per layer-1 stream tile
    T2=1024,      # edge slots per layer-2 gather tile (ucode ring: <=1024)
    SLAB=4,       # layer-1 stream tiles per DMA (bigger descriptors)
    K=2,          # h1 table splits (pipelined all-gathers / layer-2 passes)
    SPLIT0=30,    # groups in table 0 (pass-0 share of layer-2 edges)
)


def _derive(cfg):
    c = dict(cfg)
    c["NPC"] = c["N"] // c["CORES"]              # nodes per core (logical)
    c["NPCP"] = -(-c["NPC"] // 128) * 128        # padded to node groups
    c["NT"] = c["NPCP"] // 128                   # node groups per core
    c["DP"] = 128                                # padded feature dim
    c["CPT1"] = c["T1"] // 128                   # chunks per layer-1 tile
    c["CPT2"] = c["T2"] // 128                   # chunks per layer-2 tile
    K = c["K"]
    if K == 2 and c.get("SPLIT0"):
        nts = [c["SPLIT0"], c["NT"] - c["SPLIT0"]]
    else:
        base, rem = divmod(c["NT"], K)
        nts = [base + (1 if k < rem else 0) for k in range(K)]
    c["GS"] = np.concatenate([[0], np.cumsum(nts)]).tolist()  # group bounds
    c["HS"] = [n * 128 for n in nts]                          # rows per core
    for h in c["HS"]:
        assert c["CORES"] * h < 32768, "int16 gather index overflow"
    return c


def _wrap_idxs(arr, n_tiles, T):
    """int arr [n_tiles*T] -> [n_tiles, 128, T//16] int16 in the SWDGE
    wrapped layout: element (t, p, s) = arr[t*T + s*16 + p%16]."""
    w = arr.reshape(n_tiles, T // 16, 16).transpose(0, 2, 1)  # [nt, 16, S]
    return np.ascontiguousarray(np.tile(w, (1, 8, 1)).astype(np.int16))


def _chunk_schedule(cnt_by_core, CPT, floor1):
    """cnt_by_core [CORES, NT] -> uniform-across-cores chunk schedule."""
    sl = (-(-cnt_by_core // 128)).max(axis=0)
    if floor1:
        # every group needs >=1 chunk so its PSUM tile is always written
        sl = np.maximum(sl, 1)
    starts = np.concatenate([[0], np.cumsum(sl)])
    tot = max(1, int(sl.sum()))
    n_tiles = -(-tot // CPT)
    tile_cpt = [min(CPT, max(1, tot - t * CPT)) for t in range(n_tiles)]
    return sl, starts, n_tiles, tile_cpt


def _prep(inputs, cfg):
    """Host-side sharding. Returns (in_maps, meta, node2row) where
    node2row[n] is the node's row in the padded per-core layout."""
    N, D, CORES, K = cfg["N"], cfg["D"], cfg["CORES"], cfg["K"]
    NPCP, NT, DP = cfg["NPCP"], cfg["NT"], cfg["DP"]
    T1, CPT1, T2, CPT2 = cfg["T1"], cfg["CPT1"], cfg["T2"], cfg["CPT2"]
    GS, HS = cfg["GS"], cfg["HS"]

    x = np.asarray(inputs["x"], np.float32)
    x_bf = x.astype(NPBF)
    ei = np.asarray(inputs["edge_index"]).astype(np.int64)
    src, dst = ei[0], ei[1]

    # deal nodes round-robin by in-degree rank: rank r -> core r%CORES,
    # per-core slot r//CORES (group-major). Every core sees the same degree
    # sequence (edge balance), and rows within a group are degree-sorted so
    # fixed-block layer-1 chunks waste little padding.
    deg_in = np.bincount(dst, minlength=N)
    order_nodes = np.argsort(-deg_in, kind="stable")
    rank = np.empty(N, np.int64)
    rank[order_nodes] = np.arange(N)
    owner_of = rank % CORES
    q_ = rank // CORES
    assert q_.max() < NPCP
    node2row = owner_of * NPCP + q_  # global padded row

    owner = owner_of[dst]
    row_d = node2row[dst]

    # per-core dst-sorted edge lists
    per_core = []
    for c in range(CORES):
        sel = owner == c
        d = row_d[sel] - c * NPCP
        order = np.argsort(d, kind="stable")
        per_core.append((src[sel][order], d[order]))

    # ---- layer-1 fixed-block schedule ----
    # chunk pattern j covers dsts 16j..16j+15 of its group, 8 slots each;
    # slice j repeats ceil(max_deg/8) times (uniform across cores because
    # the degree deal aligns slice degrees). One constant one-hot per j.
    BS, DPC = 4, 32            # slots per dst block, dsts per chunk
    # max degree per (group, slice) over all cores
    deg_row = np.zeros(CORES * NPCP, np.int64)
    deg_row[node2row] = deg_in
    deg_row = deg_row.reshape(CORES, NT, 128 // DPC, DPC)  # [c, g, slice, dst]
    nrep = np.maximum(-(-deg_row.max(axis=(0, 3)) // BS), 1)  # [NT, 8 slices]
    pat1, grp_chunks = [], []   # pattern id per global chunk, chunks per group
    for g in range(NT):
        cl = []
        for j in range(128 // DPC):
            cl += [j] * int(nrep[g, j])
        grp_chunks.append(len(cl))
        pat1 += cl
    sl1 = np.array(grp_chunks, np.int64)
    st1 = np.concatenate([[0], np.cumsum(sl1)])
    tot1 = int(sl1.sum())
    nt1 = -(-tot1 // CPT1)
    tcpt1 = [min(CPT1, max(1, tot1 - t * CPT1)) for t in range(nt1)]

    # ---- layer-2 per-split chunk schedules ----
    perl2 = [[] for _ in range(K)]
    for c in range(CORES):
        s_, d_ = per_core[c]
        oc = owner_of[s_]
        lr = node2row[s_] - oc * NPCP
        gq = lr // 128
        for k in range(K):
            ink = (gq >= GS[k]) & (gq < GS[k + 1])
            perl2[k].append(((oc * HS[k] + lr - GS[k] * 128)[ink], d_[ink]))
    meta2 = []
    for k in range(K):
        cnt = np.zeros((CORES, NT), np.int64)
        for c in range(CORES):
            cnt[c] = np.bincount(perl2[k][c][1] // 128, minlength=NT)
        meta2.append(_chunk_schedule(cnt, CPT2, k == 0))

    in_maps = []
    shared = {}
    for nm, key in zip(["w1r", "w1n", "w2r", "w2n"],
                       ["W1_root", "W1_nbr", "W2_root", "W2_nbr"]):
        shared[nm] = np.asarray(inputs[key], np.float32).astype(NPBF)
    lin_w = np.asarray(inputs["lin_w"], np.float32)
    shared["la"] = np.ascontiguousarray(lin_w[:, :D].T).astype(NPBF)
    shared["lb"] = np.ascontiguousarray(lin_w[:, D:].T).astype(NPBF)
    shared["b1"] = np.asarray(inputs["b1"], np.float32).reshape(D, 1)
    shared["b1r"] = np.asarray(inputs["b1"], np.float32).reshape(1, D).astype(NPBF)
    shared["b2"] = np.asarray(inputs["b2"], np.float32).reshape(D, 1)
    shared["lbias"] = np.asarray(inputs["lin_b"], np.float32).reshape(D, 1)
    shared["zeros"] = np.zeros((NPCP, DP), NPBF)
    # constant layer-1 one-hots: ohc[s, j, d] = (d == DPC*j + s//BS)
    s_arr = np.arange(128)[:, None, None]
    j_arr = np.arange(128 // DPC)[None, :, None]
    d_arr = np.arange(128)[None, None, :]
    shared["ohc"] = (d_arr == DPC * j_arr + s_arr // BS).astype(NPBF)

    SENT = np.float32(500.0)  # one-hot sentinel: matches no iota column
    inv_deg = (1.0 / np.maximum(deg_in, 1)).astype(np.float32)

    for c in range(CORES):
        m = dict(shared)
        xT = np.zeros((D, NPCP), NPBF)
        mine = owner_of == c
        lrow = node2row[mine] - c * NPCP
        xT[:, lrow] = x_bf[mine].T
        m["xT"] = xT
        # 1/deg for this core's nodes, [p, g] layout (pad rows -> 1.0)
        invd = np.ones((128, NT), np.float32)
        invd[lrow % 128, lrow // 128] = inv_deg[mine]
        m["invd"] = invd

        s_, d_ = per_core[c]
        bounds = np.searchsorted(d_, np.arange(NT + 1) * 128)

        # layer-1 message stream in fixed-block layout: chunk with pattern
        # j holds dst rows DPC*j + s//BS; slot s serves edge BS*rep + s%BS
        # of that dst (zero rows pad short blocks)
        SLAB = cfg["SLAB"]
        nslab = -(-nt1 // SLAB)
        msg_lin = np.zeros((nslab * SLAB * CPT1 * 128, D), NPBF)
        drow_bnd = np.searchsorted(d_, np.arange(NPCP + 1))
        for g in range(NT):
            reps_done = np.zeros(128 // DPC, np.int64)
            for ci in range(sl1[g]):
                j = pat1[st1[g] + ci]
                rep = reps_done[j]
                reps_done[j] += 1
                base = (st1[g] + ci) * 128
                for t in range(DPC):
                    drow = g * 128 + DPC * j + t
                    lo_, hi_ = drow_bnd[drow], drow_bnd[drow + 1]
                    e0 = lo_ + rep * BS
                    ne = min(BS, hi_ - e0)
                    if ne > 0:
                        sl_s = base + t * BS
                        msg_lin[sl_s:sl_s + ne, :] = x_bf[s_[e0:e0 + ne]]
        m["msg1"] = np.ascontiguousarray(
            msg_lin.reshape(nslab, SLAB * CPT1, 128, D)
            .transpose(0, 2, 1, 3).reshape(nslab, 128, SLAB * CPT1 * D))

        # layer-2 gather indices + dst values, per table split
        for k in range(K):
            sv, dv = perl2[k][c]
            sl, st, nt, _ = meta2[k]
            L = nt * T2
            si = np.zeros(L, np.int64)
            dval = np.full(L, SENT, np.float32)
            bnd = np.searchsorted(dv, np.arange(NT + 1) * 128)
            for g in range(NT):
                lo_, hi_ = bnd[g], bnd[g + 1]
                kk = hi_ - lo_
                assert kk <= sl[g] * 128, (k, g, kk, sl[g])
                pos = st[g] * 128
                si[pos:pos + kk] = sv[lo_:hi_]
                dval[pos:pos + kk] = (dv[lo_:hi_] - g * 128).astype(np.float32)
            wi = _wrap_idxs(si, nt, T2)  # [nt, 128, S]
            m[f"si_2{k}"] = np.ascontiguousarray(
                wi.transpose(1, 0, 2).reshape(128, nt * (T2 // 16)))
            m[f"dv_2{k}"] = np.ascontiguousarray(
                dval.reshape(nt, CPT2, 128).transpose(2, 0, 1)
                .reshape(128, nt * CPT2)).astype(NPBF)
        in_maps.append(m)

    meta = dict(sl1=sl1, st1=st1, nt1=nt1, tcpt1=tcpt1, meta2=meta2,
                pat1=pat1)
    return in_maps, meta, node2row


def _build(cfg, meta):
    N, D, CORES, K = cfg["N"], cfg["D"], cfg["CORES"], cfg["K"]
    NPCP, NT, DP = cfg["NPCP"], cfg["NT"], cfg["DP"]
    T1, CPT1, T2, CPT2 = cfg["T1"], cfg["CPT1"], cfg["T2"], cfg["CPT2"]
    GS, HS = cfg["GS"], cfg["HS"]
    SLAB = cfg["SLAB"]
    S2 = T2 // 16
    sl1, st1, nt1, tcpt1 = (meta[k] for k in ("sl1", "st1", "nt1", "tcpt1"))
    meta2, pat1 = meta["meta2"], meta["pat1"]
    nslab = -(-nt1 // SLAB)

    NQ = 4  # SWDGE queues, round-robined so transfers overlap
    nc = bacc.Bacc("TRN2", target_bir_lowering=False, debug=False,
                   enable_asserts=True, num_devices=CORES,
                   num_swdge_queues=NQ)

    # --- I/O ---
    xT = nc.dram_tensor("xT", [D, NPCP], BF16, kind="ExternalInput")
    invd_in = nc.dram_tensor("invd", [128, NT], F32, kind="ExternalInput")
    w_in = {nm: nc.dram_tensor(nm, [D, D], BF16, kind="ExternalInput")
            for nm in ["w1r", "w1n", "w2r", "w2n", "la", "lb"]}
    b_in = {nm: nc.dram_tensor(nm, [D, 1], F32, kind="ExternalInput")
            for nm in ["b1", "b2", "lbias"]}
    zeros_in = nc.dram_tensor("zeros", [NPCP, DP], BF16, kind="ExternalInput")
    b1r_in = nc.dram_tensor("b1r", [1, D], BF16, kind="ExternalInput")
    msg1_in = nc.dram_tensor("msg1", [nslab, 128, SLAB * CPT1 * D], BF16,
                             kind="ExternalInput")
    ohc_in = nc.dram_tensor("ohc", [128, 4, 128], BF16, kind="ExternalInput")
    si_in, dv_in = [], []
    for k in range(K):
        nt_k = meta2[k][2]
        si_in.append(nc.dram_tensor(f"si_2{k}", [128, nt_k * S2], I16,
                                    kind="ExternalInput"))
        dv_in.append(nc.dram_tensor(f"dv_2{k}", [128, nt_k * CPT2], BF16,
                                    kind="ExternalInput"))
    out_T = nc.dram_tensor("out_T", [D, NPCP], F32, kind="ExternalOutput")

    # --- internal DRAM: h1 in K third-tables so the all-gathers pipeline ---
    h1own = [nc.dram_tensor(f"h1own{k}", [HS[k], DP], BF16) for k in range(K)]
    h1full = [nc.dram_tensor(f"h1full{k}", [CORES * HS[k], DP], BF16,
                             addr_space="Shared") for k in range(K)]

    with tile.TileContext(nc) as tc:
        with tc.tile_pool(name="const", bufs=1) as const, \
             tc.tile_pool(name="resident", bufs=1) as res, \
             tc.tile_pool(name="idx", bufs=1) as idxp, \
             tc.tile_pool(name="msg", bufs=8) as msgp, \
             tc.tile_pool(name="oh", bufs=8) as ohp, \
             tc.tile_pool(name="node", bufs=8) as nodep, \
             tc.tile_pool(name="ps_g", bufs=3, space="PSUM") as ps_g, \
             tc.tile_pool(name="ps_t", bufs=2, space="PSUM") as ps_t, \
             tc.tile_pool(name="ps_mm", bufs=2, space="PSUM") as ps_mm:

            ident_bf = const.tile([128, 128], BF16, tag="id_bf")
            make_identity(nc, ident_bf[:])
            ident_f = const.tile([128, 128], F32, tag="id_f")
            make_identity(nc, ident_f[:])
            # iota_bf[p, c, j] = j -- one-hot compare target (layer-2 tiles)
            iota_i = const.tile([128, CPT2, 128], I32)
            nc.gpsimd.iota(iota_i[:], pattern=[[0, CPT2], [1, 128]], base=0,
                           channel_multiplier=0)
            iota_bf = const.tile([128, CPT2, 128], BF16)
            nc.vector.tensor_copy(iota_bf[:], iota_i[:])
            ohc_sb = const.tile([128, 4, 128], BF16, tag="ohc")
            nc.sync.dma_start(ohc_sb[:], ohc_in[:, :, :])

            qctr = [0]

            w_sb = {}
            for nm, h in w_in.items():
                w_sb[nm] = const.tile([D, D], BF16, tag=f"w_{nm}",
                                      name=f"w_{nm}")
                nc.sync.dma_start(w_sb[nm][:], h[:, :])
            b_sb = {}
            for nm, h in b_in.items():
                b_sb[nm] = const.tile([D, 1], F32, tag=f"b_{nm}",
                                      name=f"b_{nm}")
                nc.sync.dma_start(b_sb[nm][:], h[:, :])
            invd_sb = const.tile([128, NT], F32, tag="invd")
            nc.sync.dma_start(invd_sb[:], invd_in[:, :])
            b1r_sb = const.tile([1, D], BF16, tag="b1r")
            nc.sync.dma_start(b1r_sb[:], b1r_in[:, :])
            ones_sb = const.tile([1, 128], BF16, tag="ones")
            nc.vector.memset(ones_sb[:], 1.0)

            h1T_sb = res.tile([D, NPCP], BF16, tag="h1T")
            xT_sb = res.tile([D, NPCP], BF16, tag="xT_sb")
            nc.sync.dma_start(xT_sb[:], xT[:, :])

            # zero the h1 gather tables (pad cols must stay finite)
            for k in range(K):
                nc.sync.dma_start(h1own[k][:, :], zeros_in[0:HS[k], :])

            # layer-2 index/dst-value arrays, one DMA each
            idx_sb, dv2_sb = [], []
            for k in range(K):
                nt_k = meta2[k][2]
                t_ = idxp.tile([128, nt_k * S2], I16, tag=f"si2{k}",
                               name=f"si2{k}")
                nc.sync.dma_start(t_[:], si_in[k][:, :])
                idx_sb.append(t_)
                t_ = idxp.tile([128, nt_k * CPT2], BF16, tag=f"dv2{k}",
                               name=f"dv2{k}")
                nc.sync.dma_start(t_[:], dv_in[k][:, :])
                dv2_sb.append(t_)

            # ---------- layer 1: host-streamed messages ----------
            slabs = {}

            def get_slab1(sb):
                if sb not in slabs:
                    slab = msgp.tile([128, SLAB * CPT1 * D], BF16,
                                     tag="msg1", name="msg1", bufs=4)
                    # big stream slabs ride the sync HWDGE ring; the
                    # scalar engine keeps only critical-chain activations
                    nc.sync.dma_start(slab[:], msg1_in[sb, :, :])
                    slabs[sb] = slab
                return slabs[sb]

            def l1_group(g):
                psg = ps_g.tile([128, 128], F32, tag="grp", name="grp")
                chunks = [st1[g] + j for j in range(sl1[g])]
                for ci, ch in enumerate(chunks):
                    ti, kk = divmod(int(ch), CPT1)
                    sb, sub = divmod(ti, SLAB)
                    slab = get_slab1(sb)
                    off = (sub * CPT1 + kk) * D
                    # psum [node, feat] = const-onehot.T @ msg (96-col rhs
                    # keeps the host stream unpadded)
                    nc.tensor.matmul(psg[:, :D], ohc_sb[:, pat1[int(ch)], :],
                                     slab[:, off:off + D],
                                     start=ci == 0, stop=ci == len(chunks) - 1)
                sl = slice(g * 128, (g + 1) * 128)
                a_nm = nodep.tile([128, D], BF16, tag="a_nm", name="a_nm")
                nc.scalar.activation(a_nm[:], psg[:, :D],
                                     mybir.ActivationFunctionType.Identity)
                a_ps = ps_t.tile([128, 128], BF16, tag="tr2", name="tr2", bufs=1)
                nc.tensor.transpose(a_ps[:D, :], a_nm[:, :D], ident_bf[:])
                aT_bf = nodep.tile([D, 128], BF16, tag="aT_bf", name="aT_bf")
                nc.vector.tensor_copy(aT_bf[:], a_ps[:D, :])
                # node-major h1 for the gather table: the critical chain to
                # the all-gather is psum -> aT_bf -> 3 matmuls -> h_nm -> DMA
                hn = ps_t.tile([128, 128], F32, tag="hn", name="hn", bufs=1)
                nc.tensor.matmul(hn[:, :D], xT_sb[:, sl], w_sb["w1r"][:],
                                 start=True, stop=False)
                nc.tensor.matmul(hn[:, :D], aT_bf[:], w_sb["w1n"][:],
                                 start=False, stop=False)
                nc.tensor.matmul(hn[:, :D], ones_sb[:, :], b1r_sb[:, :],
                                 start=False, stop=True)
                h_nm = nodep.tile([128, D], BF16, tag="h_nm", name="h_nm")
                nc.scalar.activation(h_nm[:], hn[:, :D],
                                     mybir.ActivationFunctionType.Identity)
                # feature-major h1 (layer-2 dense + head), off the chain
                hps = ps_mm.tile([D, 128], F32, tag="mm", name="mm", bufs=1)
                nc.tensor.matmul(hps[:], w_sb["w1r"][:], xT_sb[:, sl],
                                 start=True, stop=False)
                nc.tensor.matmul(hps[:], w_sb["w1n"][:], aT_bf[:],
                                 start=False, stop=True)
                nc.scalar.activation(h1T_sb[:, sl], hps[:],
                                     mybir.ActivationFunctionType.Identity,
                                     bias=b_sb["b1"][:, 0:1])
                k = next(i for i in range(K) if g < GS[i + 1])
                slK = slice(g * 128 - GS[k] * 128, (g + 1) * 128 - GS[k] * 128)
                nc.sync.dma_start(h1own[k][slK, 0:D], h_nm[:])
                # fire each third-table's all-gather as soon as it is
                # complete, overlapping it with the rest of layer 1
                if g == GS[k + 1] - 1:
                    nc.gpsimd.collective_compute(
                        "AllGather", mybir.AluOpType.bypass,
                        replica_groups=[list(range(CORES))],
                        ins=[h1own[k].ap()], outs=[h1full[k].ap()],
                    )

            # ---------- layer 2: SWDGE gathers from the bf16 tables ----------
            state2 = [dict() for _ in range(K)]

            def get_tile2(k, ti):
                st = state2[k]
                if ti not in st:
                    cpt_t = meta2[k][3][ti]
                    nv = cpt_t * 128
                    si = idx_sb[k][:, ti * S2:ti * S2 + nv // 16]
                    dv = dv2_sb[k][:, ti * CPT2:ti * CPT2 + cpt_t]
                    msg = msgp.tile([128, CPT2, DP], BF16, tag="msg2",
                                    name="msg2", bufs=24)
                    q = qctr[0] % NQ
                    qctr[0] += 1
                    nc.gpsimd.dma_gather(msg[:, :cpt_t],
                                         h1full[k][0:CORES * HS[k], :], si,
                                         nv, nv, DP, elem_step=DP,
                                         queue_num=q)
                    oh = ohp.tile([128, CPT2, 128], BF16, tag="oh2",
                                  name="oh2")
                    nc.vector.tensor_tensor(
                        out=oh[:, :cpt_t], in0=iota_bf[:, :cpt_t],
                        in1=dv.to_broadcast([128, cpt_t, 128]),
                        op=mybir.AluOpType.is_equal)
                    st[ti] = (msg, oh)
                return st[ti]

            partial = res.tile([128, NT * 128], BF16, tag="partial")

            def part_post(g, psg):
                nc.scalar.activation(partial[:, g * 128:(g + 1) * 128],
                                     psg[:],
                                     mybir.ActivationFunctionType.Identity)

            def l2_post(g, psg):
                sl = slice(g * 128, (g + 1) * 128)
                a_bf = nodep.tile([128, DP], BF16, tag="a_bf", name="a_bf")
                nc.scalar.activation(a_bf[:], psg[:],
                                     mybir.ActivationFunctionType.Identity,
                                     scale=invd_sb[:, g:g + 1])
                a_ps = ps_t.tile([128, 128], BF16, tag="tr", name="tr", bufs=1)
                nc.tensor.transpose(a_ps[:], a_bf[:], ident_bf[:])
                aT = nodep.tile([D, 128], BF16, tag="aT2", name="aT2")
                nc.vector.tensor_copy(aT[:], a_ps[:D, :])

                hps = ps_mm.tile([D, 128], F32, tag="mm", name="mm", bufs=1)
                nc.tensor.matmul(hps[:], w_sb["w2r"][:], h1T_sb[:, sl],
                                 start=True, stop=False)
                nc.tensor.matmul(hps[:], w_sb["w2n"][:], aT[:],
                                 start=False, stop=True)
                h2T = nodep.tile([D, 128], BF16, tag="h2T", name="h2T")
                nc.scalar.activation(h2T[:], hps[:],
                                     mybir.ActivationFunctionType.Identity,
                                     bias=b_sb["b2"][:, 0:1])

                ops = ps_mm.tile([D, 128], F32, tag="mm_out", name="mm_out",
                                 bufs=1)
                nc.tensor.matmul(ops[:], w_sb["la"][:], h1T_sb[:, sl],
                                 start=True, stop=False)
                nc.tensor.matmul(ops[:], w_sb["lb"][:], h2T[:],
                                 start=False, stop=True)
                oT = nodep.tile([D, 128], F32, tag="oT", name="oT")
                nc.scalar.activation(oT[:], ops[:],
                                     mybir.ActivationFunctionType.Relu,
                                     bias=b_sb["lbias"][:, 0:1])
                nc.sync.dma_start(out_T[:, sl], oT[:])

            def l2_group(k, g, inject, post):
                sl_k, st_k = meta2[k][0], meta2[k][1]
                psg = ps_g.tile([128, 128], F32, tag="grp", name="grp")
                chunks = [st_k[g] + j for j in range(sl_k[g])]
                nch = len(chunks) + (1 if inject else 0)
                if inject:
                    # identity matmul accumulates the prior partial sum
                    nc.tensor.matmul(psg[:], ident_bf[:],
                                     partial[:, g * 128:(g + 1) * 128],
                                     start=True, stop=nch == 1)
                for ci, ch in enumerate(chunks):
                    ti, kk = divmod(int(ch), CPT2)
                    msg, oh = get_tile2(k, ti)
                    first = ci == 0 and not inject
                    last = ci == len(chunks) - 1
                    # psum [node, feat] = onehot.T @ msg
                    nc.tensor.matmul(psg[:], oh[:, kk, :], msg[:, kk, :],
                                     start=first, stop=last)
                post(g, psg)

            # ---------- emission schedule ----------
            # layer 1 in full (its all-gathers fire per third), then the K
            # layer-2 passes; deep msg2 buffering lets pass-0 gathers run
            # ~16 tiles ahead while the PE drains the tail of layer 1
            for g in range(NT):
                l1_group(g)
            for k in range(K):
                post = l2_post if k == K - 1 else part_post
                for g in range(NT):
                    l2_group(k, g, k > 0, post)

    nc.compile()
    return nc


def build_and_run(inputs, cfg=None, trace=False, **run_kwargs):
    cfg = _derive(cfg or DEFAULT_CFG)
    in_maps, meta, node2row = _prep(inputs, cfg)
    nc = _build(cfg, meta)
    res = run_bass_kernel_spmd(nc, in_maps, list(range(cfg["CORES"])),
                               trace=trace, **run_kwargs)
    N, NPCP, D = cfg["N"], cfg["NPCP"], cfg["D"]
    out = np.empty((N, D), np.float32)
    owner_of = node2row // NPCP
    local = node2row - owner_of * NPCP
    for c in range(cfg["CORES"]):
        mine = owner_of == c
        out[mine] = res.results[c]["out_T"][:, local[mine]].T
    return out, res


def kernel(**inputs) -> np.ndarray:
    out, _ = build_and_run(inputs)
    return out


# revision 65
# speedup vs baseline: 1.0560x; 1.0156x over previous
"""Trainium2 Bass kernel for a 2-layer GNN message-passing block (SAGE-style).

Computation (see reference):
    h1 = x @ W1_root + seg_sum(x[src], dst) @ W1_nbr + b1
    a2 = seg_sum(h1[src], dst) / max(deg, 1)
    h2 = h1 @ W2_root + a2 @ W2_nbr + b2
    out = relu(h1 @ lin_w[:, :D].T + h2 @ lin_w[:, D:].T + lin_b)

Sharding: nodes are dealt to (core, 128-node group) slots in descending
in-degree order (snake), which balances per-group edge counts across cores;
edges are sharded by destination so the segment reduction is device-local.

Profiling-driven design notes (the original kernel spent ~73% of its span
on SWDGE descriptor generation; the sustained dma_gather floor is ~2.5us
per 1024-descriptor instruction, DMA-round-trip bound with 4 queues):
  * Layer-1 messages x[src] are a pure gather of the *input*, so they are
    materialized host-side as a dst-sorted bf16 stream (96 cols, unpadded
    — the stream DMA paces the layer-1 phase) and read with sequential
    DMA in 4-tile slabs — no SWDGE descriptors for layer 1.
  * Everything on the PE runs in bf16 (f32 PSUM accumulation); gather
    tables are bf16 so layer-2 SWDGE gathers move 256B/edge.
  * Layer-2 keeps the SWDGE dma_gather path (h1 is device-computed): 4
    queues round-robin, edges host-sorted by destination, segment sum via
    one-hot matmuls into one PSUM tile per node group (dma_scatter_add
    loses colliding row updates on HW, so no scatter is used). 1/deg is
    a pure function of edge_index, so it is computed host-side.
  * Layer-1's one-hots are *constant*: nodes are dealt round-robin by
    in-degree rank, so rows within a group are degree-sorted and the
    host packs each dst's edges into fixed 4-slot blocks (32 dsts per
    chunk; ~8% slot padding). One constant pattern tile (the FWL
    stationary) replaces all per-tile DVE is_equal builds; the h1 table
    write is computed node-major straight from aT via two extra matmuls
    plus a rank-1 ones x b1 bias matmul, so no PE transpose sits on the
    all-gather-critical chain.
  * h1 lives in two tables (30/19 groups; the int16 gather-index limit
    caps a table at 31 groups x 8 cores). Each AllGather fires as soon
    as its table is written; the asymmetric split makes pass 0 long
    enough to hide the second AllGather. Layer 2 runs as K=2 passes;
    pass-0 partial sums park in SBUF (bf16) and are injected back into
    PSUM via an identity matmul during pass 1. Deep (24-buffer) msg
    tiles let gathers run well ahead of their consuming matmuls.
  * All index/dst-value arrays are uploaded in partition-major layouts
    and loaded with one big DMA each.

Dense math runs feature-major: weights load as stationary lhsT once and
node columns stream as rhs. The final output is produced transposed and
scattered back to original node order on the host.
"""
import sys

sys.path.insert(0, "/opt/trn_rl_repo")

import numpy as np
import ml_dtypes

import concourse.bass as bass
import concourse.mybir as mybir
from concourse import bacc, tile
from concourse.bass_utils import run_bass_kernel_spmd
from concourse.masks import make_identity

F32 = mybir.dt.float32
BF16 = mybir.dt.bfloat16
I16 = mybir.dt.int16
I32 = mybir.dt.int32
NPBF = ml_dtypes.bfloat16

DEFAULT_CFG = dict(
    N=50000,      # nodes
    D=96,         # feature dim
    CORES=8,
    T1=2048,      # edge slots per layer-1 stream tile
    T2=1024,      # edge slots per layer-2 gather tile (ucode ring: <=1024)
    SLAB=4,       # layer-1 stream tiles per DMA (bigger descriptors)
    K=2,          # h1 table splits (pipelined all-gathers / layer-2 passes)
    SPLIT0=30,    # groups in table 0 (pass-0 share of layer-2 edges)
)


def _derive(cfg):
    c = dict(cfg)
    c["NPC"] = c["N"] // c["CORES"]              # nodes per core (logical)
    c["NPCP"] = -(-c["NPC"] // 128) * 128        # padded to node groups
    c["NT"] = c["NPCP"] // 128                   # node groups per core
    c["DP"] = 128                                # padded feature dim
    c["CPT1"] = c["T1"] // 128                   # chunks per layer-1 tile
    c["CPT2"] = c["T2"] // 128                   # chunks per layer-2 tile
    K = c["K"]
    if K == 2 and c.get("SPLIT0"):
        nts = [c["SPLIT0"], c["NT"] - c["SPLIT0"]]
    else:
        base, rem = divmod(c["NT"], K)
        nts = [base + (1 if k < rem else 0) for k in range(K)]
    c["GS"] = np.concatenate([[0], np.cumsum(nts)]).tolist()  # group bounds
    c["HS"] = [n * 128 for n in nts]                          # rows per core
    for h in c["HS"]:
        assert c["CORES"] * h < 32768, "int16 gather index overflow"
    return c


def _wrap_idxs(arr, n_tiles, T):
    """int arr [n_tiles*T] -> [n_tiles, 128, T//16] int16 in the SWDGE
    wrapped layout: element (t, p, s) = arr[t*T + s*16 + p%16]."""
    w = arr.reshape(n_tiles, T // 16, 16).transpose(0, 2, 1)  # [nt, 16, S]
    return np.ascontiguousarray(np.tile(w, (1, 8, 1)).astype(np.int16))


def _chunk_schedule(cnt_by_core, CPT, floor1):
    """cnt_by_core [CORES, NT] -> uniform-across-cores chunk schedule."""
    sl = (-(-cnt_by_core // 128)).max(axis=0)
    if floor1:
        # every group needs >=1 chunk so its PSUM tile is always written
        sl = np.maximum(sl, 1)
    starts = np.concatenate([[0], np.cumsum(sl)])
    tot = max(1, int(sl.sum()))
    n_tiles = -(-tot // CPT)
    tile_cpt = [min(CPT, max(1, tot - t * CPT)) for t in range(n_tiles)]
    return sl, starts, n_tiles, tile_cpt


def _prep(inputs, cfg):
    """Host-side sharding. Returns (in_maps, meta, node2row) where
    node2row[n] is the node's row in the padded per-core layout."""
    N, D, CORES, K = cfg["N"], cfg["D"], cfg["CORES"], cfg["K"]
    NPCP, NT, DP = cfg["NPCP"], cfg["NT"], cfg["DP"]
    T1, CPT1, T2, CPT2 = cfg["T1"], cfg["CPT1"], cfg["T2"], cfg["CPT2"]
    GS, HS = cfg["GS"], cfg["HS"]

    x = np.asarray(inputs["x"], np.float32)
    x_bf = x.astype(NPBF)
    ei = np.asarray(inputs["edge_index"]).astype(np.int64)
    src, dst = ei[0], ei[1]

    # deal nodes round-robin by in-degree rank: rank r -> core r%CORES,
    # per-core slot r//CORES (group-major). Every core sees the same degree
    # sequence (edge balance), and rows within a group are degree-sorted so
    # fixed-block layer-1 chunks waste little padding.
    deg_in = np.bincount(dst, minlength=N)
    order_nodes = np.argsort(-deg_in, kind="stable")
    rank = np.empty(N, np.int64)
    rank[order_nodes] = np.arange(N)
    owner_of = rank % CORES
    q_ = rank // CORES
    assert q_.max() < NPCP
    node2row = owner_of * NPCP + q_  # global padded row

    owner = owner_of[dst]
    row_d = node2row[dst]

    # per-core dst-sorted edge lists
    per_core = []
    for c in range(CORES):
        sel = owner == c
        d = row_d[sel] - c * NPCP
        order = np.argsort(d, kind="stable")
        per_core.append((src[sel][order], d[order]))

    # ---- layer-1 fixed-block schedule ----
    # chunk pattern j covers dsts 16j..16j+15 of its group, 8 slots each;
    # slice j repeats ceil(max_deg/8) times (uniform across cores because
    # the degree deal aligns slice degrees). One constant one-hot per j.
    BS, DPC = 4, 32            # slots per dst block, dsts per chunk
    # max degree per (group, slice) over all cores
    deg_row = np.zeros(CORES * NPCP, np.int64)
    deg_row[node2row] = deg_in
    deg_row = deg_row.reshape(CORES, NT, 128 // DPC, DPC)  # [c, g, slice, dst]
    nrep = np.maximum(-(-deg_row.max(axis=(0, 3)) // BS), 1)  # [NT, 8 slices]
    pat1, grp_chunks = [], []   # pattern id per global chunk, chunks per group
    for g in range(NT):
        cl = []
        for j in range(128 // DPC):
            cl += [j] * int(nrep[g, j])
        grp_chunks.append(len(cl))
        pat1 += cl
    sl1 = np.array(grp_chunks, np.int64)
    st1 = np.concatenate([[0], np.cumsum(sl1)])
    tot1 = int(sl1.sum())
    nt1 = -(-tot1 // CPT1)
    tcpt1 = [min(CPT1, max(1, tot1 - t * CPT1)) for t in range(nt1)]

    # ---- layer-2 per-split chunk schedules ----
    perl2 = [[] for _ in range(K)]
    for c in range(CORES):
        s_, d_ = per_core[c]
        oc = owner_of[s_]
        lr = node2row[s_] - oc * NPCP
        gq = lr // 128
        for k in range(K):
            ink = (gq >= GS[k]) & (gq < GS[k + 1])
            perl2[k].append(((oc * HS[k] + lr - GS[k] * 128)[ink], d_[ink]))
    meta2 = []
    for k in range(K):
        cnt = np.zeros((CORES, NT), np.int64)
        for c in range(CORES):
            cnt[c] = np.bincount(perl2[k][c][1] // 128, minlength=NT)
        meta2.append(_chunk_schedule(cnt, CPT2, k == 0))

    in_maps = []
    shared = {}
    for nm, key in zip(["w1r", "w1n", "w2r", "w2n"],
                       ["W1_root", "W1_nbr", "W2_root", "W2_nbr"]):
        shared[nm] = np.asarray(inputs[key], np.float32).astype(NPBF)
    lin_w = np.asarray(inputs["lin_w"], np.float32)
    shared["la"] = np.ascontiguousarray(lin_w[:, :D].T).astype(NPBF)
    shared["lb"] = np.ascontiguousarray(lin_w[:, D:].T).astype(NPBF)
    shared["b1"] = np.asarray(inputs["b1"], np.float32).reshape(D, 1)
    shared["b1r"] = np.asarray(inputs["b1"], np.float32).reshape(1, D).astype(NPBF)
    shared["b2"] = np.asarray(inputs["b2"], np.float32).reshape(D, 1)
    shared["lbias"] = np.asarray(inputs["lin_b"], np.float32).reshape(D, 1)
    shared["zeros"] = np.zeros((NPCP, DP), NPBF)
    # constant layer-1 one-hots: ohc[s, j, d] = (d == DPC*j + s//BS)
    s_arr = np.arange(128)[:, None, None]
    j_arr = np.arange(128 // DPC)[None, :, None]
    d_arr = np.arange(128)[None, None, :]
    shared["ohc"] = (d_arr == DPC * j_arr + s_arr // BS).astype(NPBF)

    SENT = np.float32(500.0)  # one-hot sentinel: matches no iota column
    inv_deg = (1.0 / np.maximum(deg_in, 1)).astype(np.float32)

    for c in range(CORES):
        m = dict(shared)
        xT = np.zeros((D, NPCP), NPBF)
        mine = owner_of == c
        lrow = node2row[mine] - c * NPCP
        xT[:, lrow] = x_bf[mine].T
        m["xT"] = xT
        # 1/deg for this core's nodes, [p, g] layout (pad rows -> 1.0)
        invd = np.ones((128, NT), np.float32)
        invd[lrow % 128, lrow // 128] = inv_deg[mine]
        m["invd"] = invd

        s_, d_ = per_core[c]
        bounds = np.searchsorted(d_, np.arange(NT + 1) * 128)

        # layer-1 message stream in fixed-block layout: chunk with pattern
        # j holds dst rows DPC*j + s//BS; slot s serves edge BS*rep + s%BS
        # of that dst (zero rows pad short blocks)
        SLAB = cfg["SLAB"]
        nslab = -(-nt1 // SLAB)
        msg_lin = np.zeros((nslab * SLAB * CPT1 * 128, D), NPBF)
        drow_bnd = np.searchsorted(d_, np.arange(NPCP + 1))
        for g in range(NT):
            reps_done = np.zeros(128 // DPC, np.int64)
            for ci in range(sl1[g]):
                j = pat1[st1[g] + ci]
                rep = reps_done[j]
                reps_done[j] += 1
                base = (st1[g] + ci) * 128
                for t in range(DPC):
                    drow = g * 128 + DPC * j + t
                    lo_, hi_ = drow_bnd[drow], drow_bnd[drow + 1]
                    e0 = lo_ + rep * BS
                    ne = min(BS, hi_ - e0)
                    if ne > 0:
                        sl_s = base + t * BS
                        msg_lin[sl_s:sl_s + ne, :] = x_bf[s_[e0:e0 + ne]]
        m["msg1"] = np.ascontiguousarray(
            msg_lin.reshape(nslab, SLAB * CPT1, 128, D)
            .transpose(0, 2, 1, 3).reshape(nslab, 128, SLAB * CPT1 * D))

        # layer-2 gather indices + dst values, per table split
        for k in range(K):
            sv, dv = perl2[k][c]
            sl, st, nt, _ = meta2[k]
            L = nt * T2
            si = np.zeros(L, np.int64)
            dval = np.full(L, SENT, np.float32)
            bnd = np.searchsorted(dv, np.arange(NT + 1) * 128)
            for g in range(NT):
                lo_, hi_ = bnd[g], bnd[g + 1]
                kk = hi_ - lo_
                assert kk <= sl[g] * 128, (k, g, kk, sl[g])
                pos = st[g] * 128
                si[pos:pos + kk] = sv[lo_:hi_]
                dval[pos:pos + kk] = (dv[lo_:hi_] - g * 128).astype(np.float32)
            wi = _wrap_idxs(si, nt, T2)  # [nt, 128, S]
            m[f"si_2{k}"] = np.ascontiguousarray(
                wi.transpose(1, 0, 2).reshape(128, nt * (T2 // 16)))
            m[f"dv_2{k}"] = np.ascontiguousarray(
                dval.reshape(nt, CPT2, 128).transpose(2, 0, 1)
                .reshape(128, nt * CPT2)).astype(NPBF)
        in_maps.append(m)

    meta = dict(sl1=sl1, st1=st1, nt1=nt1, tcpt1=tcpt1, meta2=meta2,
                pat1=pat1)
    return in_maps, meta, node2row


def _build(cfg, meta):
    N, D, CORES, K = cfg["N"], cfg["D"], cfg["CORES"], cfg["K"]
    NPCP, NT, DP = cfg["NPCP"], cfg["NT"], cfg["DP"]
    T1, CPT1, T2, CPT2 = cfg["T1"], cfg["CPT1"], cfg["T2"], cfg["CPT2"]
    GS, HS = cfg["GS"], cfg["HS"]
    SLAB = cfg["SLAB"]
    S2 = T2 // 16
    sl1, st1, nt1, tcpt1 = (meta[k] for k in ("sl1", "st1", "nt1", "tcpt1"))
    meta2, pat1 = meta["meta2"], meta["pat1"]
    nslab = -(-nt1 // SLAB)

    NQ = 4  # SWDGE queues, round-robined so transfers overlap
    nc = bacc.Bacc("TRN2", target_bir_lowering=False, debug=False,
                   enable_asserts=True, num_devices=CORES,
                   num_swdge_queues=NQ)

    # --- I/O ---
    xT = nc.dram_tensor("xT", [D, NPCP], BF16, kind="ExternalInput")
    invd_in = nc.dram_tensor("invd", [128, NT], F32, kind="ExternalInput")
    w_in = {nm: nc.dram_tensor(nm, [D, D], BF16, kind="ExternalInput")
            for nm in ["w1r", "w1n", "w2r", "w2n", "la", "lb"]}
    b_in = {nm: nc.dram_tensor(nm, [D, 1], F32, kind="ExternalInput")
            for nm in ["b1", "b2", "lbias"]}
    zeros_in = nc.dram_tensor("zeros", [NPCP, DP], BF16, kind="ExternalInput")
    b1r_in = nc.dram_tensor("b1r", [1, D], BF16, kind="ExternalInput")
    msg1_in = nc.dram_tensor("msg1", [nslab, 128, SLAB * CPT1 * D], BF16,
                             kind="ExternalInput")
    ohc_in = nc.dram_tensor("ohc", [128, 4, 128], BF16, kind="ExternalInput")
    si_in, dv_in = [], []
    for k in range(K):
        nt_k = meta2[k][2]
        si_in.append(nc.dram_tensor(f"si_2{k}", [128, nt_k * S2], I16,
                                    kind="ExternalInput"))
        dv_in.append(nc.dram_tensor(f"dv_2{k}", [128, nt_k * CPT2], BF16,
                                    kind="ExternalInput"))
    out_T = nc.dram_tensor("out_T", [D, NPCP], F32, kind="ExternalOutput")

    # --- internal DRAM: h1 in K third-tables so the all-gathers pipeline ---
    h1own = [nc.dram_tensor(f"h1own{k}", [HS[k], DP], BF16) for k in range(K)]
    h1full = [nc.dram_tensor(f"h1full{k}", [CORES * HS[k], DP], BF16,
                             addr_space="Shared") for k in range(K)]

    with tile.TileContext(nc) as tc:
        with tc.tile_pool(name="const", bufs=1) as const, \
             tc.tile_pool(name="resident", bufs=1) as res, \
             tc.tile_pool(name="idx", bufs=1) as idxp, \
             tc.tile_pool(name="msg", bufs=8) as msgp, \
             tc.tile_pool(name="oh", bufs=8) as ohp, \
             tc.tile_pool(name="node", bufs=8) as nodep, \
             tc.tile_pool(name="ps_g", bufs=3, space="PSUM") as ps_g, \
             tc.tile_pool(name="ps_t", bufs=2, space="PSUM") as ps_t, \
             tc.tile_pool(name="ps_mm", bufs=2, space="PSUM") as ps_mm:

            ident_bf = const.tile([128, 128], BF16, tag="id_bf")
            make_identity(nc, ident_bf[:])
            ident_f = const.tile([128, 128], F32, tag="id_f")
            make_identity(nc, ident_f[:])
            # iota_bf[p, c, j] = j -- one-hot compare target (layer-2 tiles)
            iota_i = const.tile([128, CPT2, 128], I32)
            nc.gpsimd.iota(iota_i[:], pattern=[[0, CPT2], [1, 128]], base=0,
                           channel_multiplier=0)
            iota_bf = const.tile([128, CPT2, 128], BF16)
            nc.vector.tensor_copy(iota_bf[:], iota_i[:])
            ohc_sb = const.tile([128, 4, 128], BF16, tag="ohc")
            nc.sync.dma_start(ohc_sb[:], ohc_in[:, :, :])

            qctr = [0]

            w_sb = {}
            for nm, h in w_in.items():
                w_sb[nm] = const.tile([D, D], BF16, tag=f"w_{nm}",
                                      name=f"w_{nm}")
                nc.sync.dma_start(w_sb[nm][:], h[:, :])
            b_sb = {}
            for nm, h in b_in.items():
                b_sb[nm] = const.tile([D, 1], F32, tag=f"b_{nm}",
                                      name=f"b_{nm}")
                nc.sync.dma_start(b_sb[nm][:], h[:, :])
            invd_sb = const.tile([128, NT], F32, tag="invd")
            nc.sync.dma_start(invd_sb[:], invd_in[:, :])
            b1r_sb = const.tile([1, D], BF16, tag="b1r")
            nc.sync.dma_start(b1r_sb[:], b1r_in[:, :])
            ones_sb = const.tile([1, 128], BF16, tag="ones")
            nc.vector.memset(ones_sb[:], 1.0)

            h1T_sb = res.tile([D, NPCP], BF16, tag="h1T")
            xT_sb = res.tile([D, NPCP], BF16, tag="xT_sb")
            nc.sync.dma_start(xT_sb[:], xT[:, :])

            # zero the h1 gather tables (pad cols must stay finite)
            for k in range(K):
                nc.sync.dma_start(h1own[k][:, :], zeros_in[0:HS[k], :])

            # layer-2 index/dst-value arrays, one DMA each
            idx_sb, dv2_sb = [], []
            for k in range(K):
                nt_k = meta2[k][2]
                t_ = idxp.tile([128, nt_k * S2], I16, tag=f"si2{k}",
                               name=f"si2{k}")
                nc.sync.dma_start(t_[:], si_in[k][:, :])
                idx_sb.append(t_)
                t_ = idxp.tile([128, nt_k * CPT2], BF16, tag=f"dv2{k}",
                               name=f"dv2{k}")
                nc.sync.dma_start(t_[:], dv_in[k][:, :])
                dv2_sb.append(t_)

            # ---------- layer 1: host-streamed messages ----------
            slabs = {}

            def get_slab1(sb):
                if sb not in slabs:
                    slab = msgp.tile([128, SLAB * CPT1 * D], BF16,
                                     tag="msg1", name="msg1", bufs=4)
                    # big stream slabs ride the sync HWDGE ring; the
                    # scalar engine keeps only critical-chain activations
                    nc.sync.dma_start(slab[:], msg1_in[sb, :, :])
                    slabs[sb] = slab
                return slabs[sb]

            def l1_group(g):
                psg = ps_g.tile([128, 128], F32, tag="grp", name="grp")
                chunks = [st1[g] + j for j in range(sl1[g])]
                for ci, ch in enumerate(chunks):
                    ti, kk = divmod(int(ch), CPT1)
                    sb, sub = divmod(ti, SLAB)
                    slab = get_slab1(sb)
                    off = (sub * CPT1 + kk) * D
                    # psum [node, feat] = const-onehot.T @ msg (96-col rhs
                    # keeps the host stream unpadded)
                    nc.tensor.matmul(psg[:, :D], ohc_sb[:, pat1[int(ch)], :],
                                     slab[:, off:off + D],
                                     start=ci == 0, stop=ci == len(chunks) - 1)
                sl = slice(g * 128, (g + 1) * 128)
                a_nm = nodep.tile([128, D], BF16, tag="a_nm", name="a_nm")
                nc.scalar.activation(a_nm[:], psg[:, :D],
                                     mybir.ActivationFunctionType.Identity)
                a_ps = ps_t.tile([128, 128], BF16, tag="tr2", name="tr2", bufs=1)
                nc.tensor.transpose(a_ps[:D, :], a_nm[:, :D], ident_bf[:])
                aT_bf = nodep.tile([D, 128], BF16, tag="aT_bf", name="aT_bf")
                nc.vector.tensor_copy(aT_bf[:], a_ps[:D, :])
                # node-major h1 for the gather table: the critical chain to
                # the all-gather is psum -> aT_bf -> 3 matmuls -> h_nm -> DMA
                hn = ps_t.tile([128, 128], F32, tag="hn", name="hn", bufs=1)
                nc.tensor.matmul(hn[:, :D], xT_sb[:, sl], w_sb["w1r"][:],
                                 start=True, stop=False)
                nc.tensor.matmul(hn[:, :D], aT_bf[:], w_sb["w1n"][:],
                                 start=False, stop=False)
                nc.tensor.matmul(hn[:, :D], ones_sb[:, :], b1r_sb[:, :],
                                 start=False, stop=True)
                h_nm = nodep.tile([128, D], BF16, tag="h_nm", name="h_nm")
                nc.scalar.activation(h_nm[:], hn[:, :D],
                                     mybir.ActivationFunctionType.Identity)
                # feature-major h1 (layer-2 dense + head), off the chain
                hps = ps_mm.tile([D, 128], F32, tag="mm", name="mm", bufs=1)
                nc.tensor.matmul(hps[:], w_sb["w1r"][:], xT_sb[:, sl],
                                 start=True, stop=False)
                nc.tensor.matmul(hps[:], w_sb["w1n"][:], aT_bf[:],
                                 start=False, stop=True)
                nc.scalar.activation(h1T_sb[:, sl], hps[:],
                                     mybir.ActivationFunctionType.Identity,
                                     bias=b_sb["b1"][:, 0:1])
                k = next(i for i in range(K) if g < GS[i + 1])
                slK = slice(g * 128 - GS[k] * 128, (g + 1) * 128 - GS[k] * 128)
                nc.sync.dma_start(h1own[k][slK, 0:D], h_nm[:])
                # fire each third-table's all-gather as soon as it is
                # complete, overlapping it with the rest of layer 1
                if g == GS[k + 1] - 1:
                    nc.gpsimd.collective_compute(
                        "AllGather", mybir.AluOpType.bypass,
                        replica_groups=[list(range(CORES))],
                        ins=[h1own[k].ap()], outs=[h1full[k].ap()],
                    )

            # ---------- layer 2: SWDGE gathers from the bf16 tables ----------
            state2 = [dict() for _ in range(K)]

            def get_tile2(k, ti):
                st = state2[k]
                if ti not in st:
                    cpt_t = meta2[k][3][ti]
                    nv = cpt_t * 128
                    si = idx_sb[k][:, ti * S2:ti * S2 + nv // 16]
                    dv = dv2_sb[k][:, ti * CPT2:ti * CPT2 + cpt_t]
                    msg = msgp.tile([128, CPT2, DP], BF16, tag="msg2",
                                    name="msg2", bufs=24)
                    q = qctr[0] % NQ
                    qctr[0] += 1
                    nc.gpsimd.dma_gather(msg[:, :cpt_t],
                                         h1full[k][0:CORES * HS[k], :], si,
                                         nv, nv, DP, elem_step=DP,
                                         queue_num=q)
                    oh = ohp.tile([128, CPT2, 128], BF16, tag="oh2",
                                  name="oh2")
                    nc.vector.tensor_tensor(
                        out=oh[:, :cpt_t], in0=iota_bf[:, :cpt_t],
                        in1=dv.to_broadcast([128, cpt_t, 128]),
                        op=mybir.AluOpType.is_equal)
                    st[ti] = (msg, oh)
                return st[ti]

            partial = res.tile([128, NT * 128], BF16, tag="partial")

            def part_post(g, psg):
                nc.scalar.activation(partial[:, g * 128:(g + 1) * 128],
                                     psg[:],
                                     mybir.ActivationFunctionType.Identity)

            def l2_post(g, psg):
                sl = slice(g * 128, (g + 1) * 128)
                a_bf = nodep.tile([128, DP], BF16, tag="a_bf", name="a_bf")
                nc.scalar.activation(a_bf[:], psg[:],
                                     mybir.ActivationFunctionType.Identity,
                                     scale=invd_sb[:, g:g + 1])
                a_ps = ps_t.tile([128, 128], BF16, tag="tr", name="tr", bufs=1)
                nc.tensor.transpose(a_ps[:], a_bf[:], ident_bf[:])
                aT = nodep.tile([D, 128], BF16, tag="aT2", name="aT2")
                nc.vector.tensor_copy(aT[:], a_ps[:D, :])

                hps = ps_mm.tile([D, 128], F32, tag="mm", name="mm", bufs=1)
                nc.tensor.matmul(hps[:], w_sb["w2r"][:], h1T_sb[:, sl],
                                 start=True, stop=False)
                nc.tensor.matmul(hps[:], w_sb["w2n"][:], aT[:],
                                 start=False, stop=True)
                h2T = nodep.tile([D, 128], BF16, tag="h2T", name="h2T")
                nc.scalar.activation(h2T[:], hps[:],
                                     mybir.ActivationFunctionType.Identity,
                                     bias=b_sb["b2"][:, 0:1])

                ops = ps_mm.tile([D, 128], F32, tag="mm_out", name="mm_out",
                                 bufs=1)
                nc.tensor.matmul(ops[:], w_sb["la"][:], h1T_sb[:, sl],
                                 start=True, stop=False)
                nc.tensor.matmul(ops[:], w_sb["lb"][:], h2T[:],
                                 start=False, stop=True)
                oT = nodep.tile([D, 128], F32, tag="oT", name="oT")
                nc.scalar.activation(oT[:], ops[:],
                                     mybir.ActivationFunctionType.Relu,
                                     bias=b_sb["lbias"][:, 0:1])
                nc.sync.dma_start(out_T[:, sl], oT[:])

            def l2_group(k, g, inject, post):
                sl_k, st_k = meta2[k][0], meta2[k][1]
                psg = ps_g.tile([128, 128], F32, tag="grp", name="grp")
                chunks = [st_k[g] + j for j in range(sl_k[g])]
                nch = len(chunks) + (1 if inject else 0)
                if inject:
                    # identity matmul accumulates the prior partial sum
                    nc.tensor.matmul(psg[:], ident_bf[:],
                                     partial[:, g * 128:(g + 1) * 128],
                                     start=True, stop=nch == 1)
                for ci, ch in enumerate(chunks):
                    ti, kk = divmod(int(ch), CPT2)
                    msg, oh = get_tile2(k, ti)
                    first = ci == 0 and not inject
                    last = ci == len(chunks) - 1
                    # psum [node, feat] = onehot.T @ msg
                    nc.tensor.matmul(psg[:], oh[:, kk, :], msg[:, kk, :],
                                     start=first, stop=last)
                post(g, psg)

            # ---------- emission schedule ----------
            # layer 1 in full (its all-gathers fire per third), then the K
            # layer-2 passes; deep msg2 buffering lets pass-0 gathers run
            # ~16 tiles ahead while the PE drains the tail of layer 1
            for g in range(NT):
                l1_group(g)
            for k in range(K):
                post = l2_post if k == K - 1 else part_post
                for g in range(NT):
                    l2_group(k, g, k > 0, post)

    nc.compile()
    return nc


def build_and_run(inputs, cfg=None, trace=False, **run_kwargs):
    cfg = _derive(cfg or DEFAULT_CFG)
    in_maps, meta, node2row = _prep(inputs, cfg)
    nc = _build(cfg, meta)
    res = run_bass_kernel_spmd(nc, in_maps, list(range(cfg["CORES"])),
                               trace=trace, **run_kwargs)
    N, NPCP, D = cfg["N"], cfg["NPCP"], cfg["D"]
    out = np.empty((N, D), np.float32)
    owner_of = node2row // NPCP
    local = node2row - owner_of * NPCP
    for c in range(cfg["CORES"]):
        mine = owner_of == c
        out[mine] = res.results[c]["out_T"][:, local[mine]].T
    return out, res


def kernel(**inputs) -> np.ndarray:
    out, _ = build_and_run(inputs)
    return out
